# revision 28
# baseline (speedup 1.0000x reference)
"""GAT (2-layer) — fully on-device Trainium2 kernel, 8 NeuronCores, one dispatch.

Design (edge-parallel over dst-sorted edges, per the sharding hint):
  - Nodes padded to NP = 50176 = 392 blocks of 128; core k owns 49 blocks.
  - Host edge plan (cached): edges sorted by dst block, each block padded to a
    uniform CPB*128 edge slots (pad slots get dst_local = -1 -> zero one-hot
    column -> no contribution).
  - Phase A (per core, own nodes): h|es1 table rows + ed1 via x @ W1aug on PE.
    AllGather -> full gather table (bf16) in device DRAM.
  - Layer loop (For_i over 49 blocks x CPB chunks of 128 edges):
      indirect DMA gathers table[src] rows (one row per partition),
      one-hot(dst_local) built with is_equal(iota, dstf),
      PE transpose of the one-hot expands per-block ed to edges,
      exp(leaky_relu(es+ed)) on ACT, message scaling on DVE,
      one-hot^T @ messages accumulates numerator+denominator in PSUM.
  - Block postprocess: normalize, bias, ELU, h2 = h1 @ W2aug -> layer-2 table.
    AllGather, same loop for layer 2, log_softmax, bf16 output per core.
"""
import sys
sys.path.insert(0, "/opt/trn_rl_repo")
import time
import numpy as np
import ml_dtypes

BF = ml_dtypes.bfloat16

N = 50000
F = 512
D1 = 64
H1, C1 = 8, 8
C2 = 40
NC = 8
NBLK = 49            # dst blocks per core
CPB = 34             # chunks (of 128 edges) per block
NP = NC * NBLK * 128  # 50176 padded nodes
NLOC = NBLK * 128     # 6272 nodes per core
NEG = 0.2

_cache = {}
device_time = [0.0]


# ---------------------------------------------------------------- bass kernel
def _build(nblk=NBLK, cpb=CPB, dbg=False, no_gather=False, no_loops=False, no_coll=False, small_coll=False):
    import concourse.bacc as bacc
    import concourse.mybir as mybir
    import concourse.tile as tile
    from concourse import bass
    from concourse.bass import ts
    from concourse.masks import make_identity

    f32 = mybir.dt.float32
    bf16 = mybir.dt.bfloat16
    i32 = mybir.dt.int32
    AF = mybir.ActivationFunctionType
    OP = mybir.AluOpType

    nloc = nblk * 128
    npad = NC * nloc

    nc = bacc.Bacc("TRN2", target_bir_lowering=False, debug=False, num_devices=NC)
    xT = nc.dram_tensor("xT", [F, nloc], f32, kind="ExternalInput")
    w1aug = nc.dram_tensor("w1aug", [F, 80], f32, kind="ExternalInput")
    w2aug = nc.dram_tensor("w2aug", [D1, 42], bf16, kind="ExternalInput")
    b1rep = nc.dram_tensor("b1rep", [128, D1], f32, kind="ExternalInput")
    b2rep = nc.dram_tensor("b2rep", [128, C2], f32, kind="ExternalInput")
    iota = nc.dram_tensor("iota", [128, 128], f32, kind="ExternalInput")
    srcs = nc.dram_tensor("srcs", [128, nblk, cpb], i32, kind="ExternalInput")
    dstf = nc.dram_tensor("dstf", [128, nblk, cpb], f32, kind="ExternalInput")
    out = nc.dram_tensor("out", [nloc, C2], bf16, kind="ExternalOutput")
    done = nc.dram_tensor("done", [1, 4], f32, kind="ExternalOutput")
    if dbg:
        d_t1 = nc.dram_tensor("d_t1", [nloc, 72], bf16, kind="ExternalOutput")
        d_ed1 = nc.dram_tensor("d_ed1", [128, nblk * H1], bf16, kind="ExternalOutput")
        d_den = nc.dram_tensor("d_den", [nloc, H1], f32, kind="ExternalOutput")
        d_h1 = nc.dram_tensor("d_h1", [nloc, D1], bf16, kind="ExternalOutput")
        d_e = nc.dram_tensor("d_e", [nloc, H1], bf16, kind="ExternalOutput")
        d_g = nc.dram_tensor("d_g", [nloc, 72], bf16, kind="ExternalOutput")
        d_sc = nc.dram_tensor("d_sc", [nloc, H1], f32, kind="ExternalOutput")
        d_oh = nc.dram_tensor("d_oh", [nloc, 128], bf16, kind="ExternalOutput")
        d_srcs = nc.dram_tensor("d_srcs", [nloc, cpb], i32, kind="ExternalOutput")
        d_tf = nc.dram_tensor("d_tf", [nloc, 72], bf16, kind="ExternalOutput")
        d_ex = nc.dram_tensor("d_ex", [nblk, cpb * 128, H1], bf16,
                              kind="ExternalOutput")

    with tile.TileContext(nc) as tc:
        with (
            tc.tile_pool(name="const", bufs=1) as cp,
            tc.tile_pool(name="dram", bufs=1, space="DRAM") as dp,
            tc.tile_pool(name="pa", bufs=3) as pa,
            tc.tile_pool(name="gp", bufs=4) as gp,
            tc.tile_pool(name="mp", bufs=4) as mp,
            tc.tile_pool(name="pp", bufs=2) as pp,
            tc.tile_pool(name="ps", bufs=2, space="PSUM") as ps,
            tc.tile_pool(name="psa", bufs=1, space="PSUM") as psa,
        ):
            # ---- constants ----
            iota_sb = cp.tile([128, 128], f32)
            nc.sync.dma_start(iota_sb[:], iota[:])
            ident = cp.tile([128, 128], bf16)
            make_identity(nc, ident[:])
            b1_sb = cp.tile([128, H1, C1], f32)
            nc.sync.dma_start(b1_sb[:], b1rep[:, :, None].rearrange(
                "p (h c) one -> p h (c one)", h=H1))
            b2_sb = cp.tile([128, C2], f32)
            nc.sync.dma_start(b2_sb[:], b2rep[:])
            w1_sb = cp.tile([128, 4, 80], f32)
            for c in range(4):
                nc.sync.dma_start(w1_sb[:, c, :], w1aug[c * 128:(c + 1) * 128, :])
            w2_sb = cp.tile([D1, 42], bf16)
            nc.sync.dma_start(w2_sb[:], w2aug[:])
            srcs_sb = cp.tile([128, nblk, cpb], i32)
            nc.sync.dma_start(srcs_sb[:], srcs[:])
            dstf_sb = cp.tile([128, nblk, cpb], f32)
            nc.sync.dma_start(dstf_sb[:], dstf[:])
            ed1_sb = cp.tile([128, nblk, H1], bf16)
            ed2_sb = cp.tile([128, nblk, 1], bf16)

            # ---- gather tables (device DRAM) ----
            t1_shard = dp.tile([nloc, 72], bf16)
            t1_full = dp.tile([npad, 72], bf16, addr_space="Shared")
            t2_shard = dp.tile([nloc, 41], bf16)
            t2_full = dp.tile([npad, 41], bf16, addr_space="Shared")

            # ---- phase A: table1 rows (h|es1) + ed1 for own nodes ----
            for b in range(nblk):
                xt = pa.tile([128, 4, 128], f32)
                for c in range(4):
                    nc.sync.dma_start(
                        xt[:, c, :],
                        xT[c * 128:(c + 1) * 128, b * 128:(b + 1) * 128])
                hps = ps.tile([128, 80], f32, space="PSUM", tag="big")
                for c in range(4):
                    nc.tensor.matmul(hps[:], lhsT=xt[:, c, :], rhs=w1_sb[:, c, :],
                                     start=(c == 0), stop=(c == 3))
                t1row = pa.tile([128, 72], bf16, tag="t1row")
                nc.vector.tensor_copy(t1row[:], hps[:, 0:72])
                nc.vector.tensor_copy(ed1_sb[:, b, :], hps[:, 72:80])
                nc.sync.dma_start(t1_shard[b * 128:(b + 1) * 128, :], t1row[:])
                if dbg:
                    nc.sync.dma_start(d_t1[b * 128:(b + 1) * 128, :], t1row[:])

            if dbg:
                nc.sync.dma_start(d_ed1[:], ed1_sb[:].rearrange("p b h -> p (b h)"))
            if small_coll:
                dumm1 = dp.tile([16, 4], f32)
                dumm1o = dp.tile([NC * 16, 4], f32, addr_space="Shared")
                nc.gpsimd.dma_start(dumm1[:], b1rep[0:16, 0:4])
                nc.gpsimd.collective_compute(
                    "AllGather", mybir.AluOpType.bypass,
                    replica_groups=[list(range(NC))],
                    ins=[dumm1[:]], outs=[dumm1o[:]])
            elif not no_coll:
                nc.gpsimd.collective_compute(
                    "AllGather", mybir.AluOpType.bypass,
                    replica_groups=[list(range(NC))],
                    ins=[t1_shard[:]], outs=[t1_full[:]])

            # ---- layer 1 edge loop ----
            if no_loops:
                nc.sync.dma_start(out[0:128, :], t1_shard[0:128, 0:C2])
            if dbg:
                tf_sb = cp.tile([128, 72], bf16)
                for b in range(nblk):
                    nc.sync.dma_start(tf_sb[:], t1_full[b * 128:(b + 1) * 128, :])
                    nc.sync.dma_start(d_tf[b * 128:(b + 1) * 128, :], tf_sb[:])
            srcs_stage1 = cp.tile([128, cpb], i32)
            ed1_stage = cp.tile([128, H1], bf16)
            loop_range1 = (0, 0 if no_loops else nblk, 1)
            with tc.For_i(*loop_range1) as i:
                nc.vector.tensor_copy(srcs_stage1[:],
                                      srcs_sb[:, ts(i, 1), :].squeeze(1))
                nc.vector.tensor_copy(ed1_stage[:],
                                      ed1_sb[:, ts(i, 1), :].squeeze(1))
                acc = psa.tile([128, H1, 9], f32, space="PSUM", tag="acc")
                for c in range(cpb):
                    G2d = gp.tile([128, H1 * 9], bf16, tag="G")
                    if no_gather:
                        nc.vector.memset(G2d[:], 0.5)
                    else:
                        nc.gpsimd.indirect_dma_start(
                            out=G2d[:], out_offset=None, in_=t1_full[:],
                            in_offset=bass.IndirectOffsetOnAxis(
                                ap=srcs_stage1[:, c:c + 1], axis=0))
                    G = G2d[:].rearrange("p (h n) -> p h n", n=9)
                    oh = gp.tile([128, 128], bf16, tag="oh")
                    nc.vector.tensor_tensor(
                        out=oh[:], in0=iota_sb[:],
                        in1=dstf_sb[:, ts(i, 1), c].to_broadcast((128, 128)),
                        op=OP.is_equal)
                    ohT_ps = ps.tile([128, 128], bf16, space="PSUM", tag="big")
                    nc.tensor.transpose(ohT_ps[:], oh[:], ident[:])
                    ohT = gp.tile([128, 128], bf16, tag="ohT")
                    nc.vector.tensor_copy(ohT[:], ohT_ps[:])
                    sc_ps = ps.tile([128, H1], f32, space="PSUM", tag="sc")
                    nc.tensor.matmul(sc_ps[:], lhsT=ohT[:],
                                     rhs=ed1_stage[:],
                                     start=True, stop=True)
                    e_sb = mp.tile([128, H1], bf16, tag="e_sb")
                    nc.vector.scalar_tensor_tensor(
                        out=e_sb[:], in0=sc_ps[:], scalar=1.0,
                        in1=G[:, :, 0], op0=OP.mult, op1=OP.add)
                    if dbg and c == 0:
                        nc.sync.dma_start(d_e[ts(i, 128), :], e_sb[:])
                        nc.sync.dma_start(d_srcs[ts(i, 128), :], srcs_stage1[:])
                        nc.sync.dma_start(d_g[ts(i, 128), :], G2d[:])
                        sc_sb_d = mp.tile([128, H1], f32, tag="sc_sb_d")
                        nc.vector.tensor_copy(sc_sb_d[:], sc_ps[:])
                        nc.sync.dma_start(d_sc[ts(i, 128), :], sc_sb_d[:])
                        nc.sync.dma_start(d_oh[ts(i, 128), :], ohT[:])
                    lr = mp.tile([128, H1], bf16, tag="lr")
                    nc.vector.scalar_tensor_tensor(
                        out=lr[:], in0=e_sb[:], scalar=NEG, in1=e_sb[:],
                        op0=OP.mult, op1=OP.max)
                    M = mp.tile([128, H1, 9], bf16, tag="M")
                    nc.scalar.activation(M[:, :, 0], lr[:], AF.Exp)
                    nc.vector.tensor_tensor(
                        out=M[:, :, 1:9], in0=G[:, :, 1:9],
                        in1=M[:, :, 0:1].to_broadcast((128, H1, 8)),
                        op=OP.mult)
                    if dbg:
                        nc.sync.dma_start(
                            d_ex[ts(i, 1), c * 128:(c + 1) * 128, :].squeeze(0),
                            M[:, :, 0])
                    nc.tensor.matmul(acc[:], lhsT=oh[:], rhs=M[:],
                                     start=(c == 0), stop=(c == cpb - 1))

                # ---- block post: h1 = elu(num/den + b1); table2 row ----
                den = pp.tile([128, H1], f32, tag="den")
                nc.vector.tensor_scalar_add(den[:], acc[:, :, 0], 1e-30)
                if dbg:
                    nc.sync.dma_start(d_den[ts(i, 128), :], den[:])
                rcp = pp.tile([128, H1], f32, tag="rcp")
                nc.vector.reciprocal(rcp[:], den[:])
                h1a = pp.tile([128, H1, C1], f32, tag="h1a")
                nc.vector.tensor_tensor(
                    out=h1a[:], in0=acc[:, :, 1:9],
                    in1=rcp[:, :, None].to_broadcast((128, H1, C1)), op=OP.mult)
                h1b = pp.tile([128, H1, C1], f32, tag="h1b")
                nc.vector.tensor_tensor(out=h1b[:], in0=h1a[:], in1=b1_sb[:],
                                        op=OP.add)
                mn = pp.tile([128, H1, C1], f32, tag="mn")
                nc.vector.tensor_scalar_min(mn[:], h1b[:], 0.0)
                em = pp.tile([128, H1, C1], f32, tag="em")
                nc.scalar.activation(em[:], mn[:], AF.Exp)
                h1f = pp.tile([128, H1, C1], bf16, tag="h1f")
                nc.vector.scalar_tensor_tensor(
                    out=h1f[:], in0=em[:], scalar=-1.0, in1=h1b[:],
                    op0=OP.add, op1=OP.max)
                if dbg:
                    nc.sync.dma_start(
                        d_h1[ts(i, 128), :], h1f[:].rearrange("p h c -> p (h c)"))
                h1T_ps = ps.tile([D1, 128], bf16, space="PSUM", tag="post")
                nc.tensor.transpose(
                    h1T_ps[:], h1f[:].rearrange("p h c -> p (h c)"), ident[:])
                h1T = pp.tile([D1, 128], bf16, tag="h1T")
                nc.vector.tensor_copy(h1T[:], h1T_ps[:])
                h2_ps = ps.tile([128, 42], f32, space="PSUM", tag="post")
                nc.tensor.matmul(h2_ps[:], lhsT=h1T[:], rhs=w2_sb[:],
                                 start=True, stop=True)
                t2row = pp.tile([128, 41], bf16, tag="t2row")
                nc.vector.tensor_copy(t2row[:], h2_ps[:, 0:41])
                nc.vector.tensor_copy(ed2_sb[:, ts(i, 1), :].squeeze(1),
                                      h2_ps[:, 41:42])
                nc.sync.dma_start(t2_shard[ts(i, 128), :], t2row[:])

            if small_coll:
                dumm2 = dp.tile([16, 4], f32)
                dumm2o = dp.tile([NC * 16, 4], f32, addr_space="Shared")
                nc.gpsimd.dma_start(dumm2[:], b1rep[0:16, 0:4])
                nc.gpsimd.collective_compute(
                    "AllGather", mybir.AluOpType.bypass,
                    replica_groups=[list(range(NC))],
                    ins=[dumm2[:]], outs=[dumm2o[:]])
            elif not no_coll:
                nc.gpsimd.collective_compute(
                    "AllGather", mybir.AluOpType.bypass,
                    replica_groups=[list(range(NC))],
                    ins=[t2_shard[:]], outs=[t2_full[:]])

            # ---- layer 2 edge loop ----
            srcs_stage2 = cp.tile([128, cpb], i32)
            ed2_stage = cp.tile([128, 1], bf16)
            loop_range2 = (0, 0 if no_loops else nblk, 1)
            with tc.For_i(*loop_range2) as i:
                nc.vector.tensor_copy(srcs_stage2[:],
                                      srcs_sb[:, ts(i, 1), :].squeeze(1))
                nc.vector.tensor_copy(ed2_stage[:],
                                      ed2_sb[:, ts(i, 1), :].squeeze(1))
                acc2 = psa.tile([128, 41], f32, space="PSUM", tag="acc")
                for c in range(cpb):
                    G2 = gp.tile([128, 41], bf16, tag="G2")
                    if no_gather:
                        nc.vector.memset(G2[:], 0.5)
                    else:
                        nc.gpsimd.indirect_dma_start(
                            out=G2[:], out_offset=None, in_=t2_full[:],
                            in_offset=bass.IndirectOffsetOnAxis(
                                ap=srcs_stage2[:, c:c + 1], axis=0))
                    oh = gp.tile([128, 128], bf16, tag="oh")
                    nc.vector.tensor_tensor(
                        out=oh[:], in0=iota_sb[:],
                        in1=dstf_sb[:, ts(i, 1), c].to_broadcast((128, 128)),
                        op=OP.is_equal)
                    ohT_ps = ps.tile([128, 128], bf16, space="PSUM", tag="big")
                    nc.tensor.transpose(ohT_ps[:], oh[:], ident[:])
                    ohT = gp.tile([128, 128], bf16, tag="ohT")
                    nc.vector.tensor_copy(ohT[:], ohT_ps[:])
                    sc2_ps = ps.tile([128, 1], f32, space="PSUM", tag="sc")
                    nc.tensor.matmul(sc2_ps[:], lhsT=ohT[:],
                                     rhs=ed2_stage[:],
                                     start=True, stop=True)
                    e2 = mp.tile([128, 1], bf16, tag="e2")
                    nc.vector.scalar_tensor_tensor(
                        out=e2[:], in0=sc2_ps[:], scalar=1.0,
                        in1=G2[:, 0:1], op0=OP.mult, op1=OP.add)
                    lr2 = mp.tile([128, 1], bf16, tag="lr2")
                    nc.vector.scalar_tensor_tensor(
                        out=lr2[:], in0=e2[:], scalar=NEG, in1=e2[:],
                        op0=OP.mult, op1=OP.max)
                    M2 = mp.tile([128, 41], bf16, tag="M2")
                    nc.scalar.activation(M2[:, 0:1], lr2[:], AF.Exp)
                    nc.vector.tensor_tensor(
                        out=M2[:, 1:41], in0=G2[:, 1:41],
                        in1=M2[:, 0:1].to_broadcast((128, 40)), op=OP.mult)
                    nc.tensor.matmul(acc2[:], lhsT=oh[:], rhs=M2[:],
                                     start=(c == 0), stop=(c == cpb - 1))

                # ---- block post: log_softmax(num/den + b2) ----
                den2 = pp.tile([128, 1], f32, tag="den2")
                nc.vector.tensor_scalar_add(den2[:], acc2[:, 0:1], 1e-30)
                rcp2 = pp.tile([128, 1], f32, tag="rcp2")
                nc.vector.reciprocal(rcp2[:], den2[:])
                z = pp.tile([128, C2], f32, tag="z")
                nc.vector.scalar_tensor_tensor(
                    out=z[:], in0=acc2[:, 1:41], scalar=rcp2[:, 0:1],
                    in1=b2_sb[:], op0=OP.mult, op1=OP.add)
                mx = pp.tile([128, 1], f32, tag="mx")
                nc.vector.tensor_reduce(mx[:], z[:], mybir.AxisListType.X, OP.max)
                zs = pp.tile([128, C2], f32, tag="zs")
                nc.vector.tensor_scalar(out=zs[:], in0=z[:], scalar1=mx[:, 0:1],
                                        scalar2=None, op0=OP.subtract)
                ez = pp.tile([128, C2], f32, tag="ez")
                se = pp.tile([128, 1], f32, tag="se")
                nc.scalar.activation(ez[:], zs[:], AF.Exp, accum_out=se[:])
                ls = pp.tile([128, 1], f32, tag="ls")
                nc.scalar.activation(ls[:], se[:], AF.Ln)
                ob = pp.tile([128, C2], bf16, tag="ob")
                nc.vector.tensor_scalar(out=ob[:], in0=zs[:], scalar1=ls[:, 0:1],
                                        scalar2=None, op0=OP.subtract)
                nc.sync.dma_start(out[ts(i, 128), :], ob[:])

            nc.sync.dma_start(done[:], zs[0:1, 0:4])

    nc.compile()
    return nc


# ---------------------------------------------------------------- host plan
def _edge_plan(src, dst, nblk=NBLK, cpb=CPB):
    nbt = NC * nblk
    blk = dst // 128
    order = np.argsort(blk, kind="stable")
    ssrc = src[order].astype(np.int32)
    sdst = dst[order].astype(np.int32)
    sblk = blk[order]
    cnt = np.bincount(sblk, minlength=nbt)
    assert cnt.max() <= cpb * 128, f"block edge count {cnt.max()} > {cpb * 128}"
    starts = np.zeros(nbt + 1, np.int64)
    np.cumsum(cnt, out=starts[1:])
    pos = np.arange(len(sdst), dtype=np.int64) - starts[sblk]
    srcs_full = np.zeros((nbt, cpb * 128), np.int32)
    dstf_full = np.full((nbt, cpb * 128), -1.0, np.float32)
    srcs_full[sblk, pos] = ssrc
    dstf_full[sblk, pos] = (sdst % 128).astype(np.float32)
    return (srcs_full.reshape(NC * nblk, cpb, 128),
            dstf_full.reshape(NC * nblk, cpb, 128))


def _pack_weights(W1, a_src1, a_dst1, W2, a_src2, a_dst2):
    w1aug = np.zeros((F, 80), np.float32)
    for h in range(H1):
        Wh = np.asarray(W1[:, 8 * h:8 * h + 8], np.float32)
        w1aug[:, 9 * h] = Wh @ np.asarray(a_src1[h], np.float32)
        w1aug[:, 9 * h + 1:9 * h + 9] = Wh
        w1aug[:, 72 + h] = Wh @ np.asarray(a_dst1[h], np.float32)
    w2aug = np.zeros((D1, 42), np.float32)
    W2 = np.asarray(W2, np.float32)
    w2aug[:, 0] = W2 @ np.asarray(a_src2[0], np.float32)
    w2aug[:, 1:41] = W2
    w2aug[:, 41] = W2 @ np.asarray(a_dst2[0], np.float32)
    return w1aug, w2aug.astype(BF)


# ---------------------------------------------------------------- jax runner
def _make_runner(nc):
    import jax
    import concourse.mybir as mybir
    from jax.sharding import Mesh, PartitionSpec
    from jax.experimental.shard_map import shard_map
    from concourse.bass2jax import (
        install_neuronx_cc_hook, _bass_exec_p, partition_id_tensor)
    install_neuronx_cc_hook()
    partition_name = nc.partition_id_tensor.name if nc.partition_id_tensor else None
    in_names, out_names, out_avals, zero_outs = [], [], [], []
    for alloc in nc.m.functions[0].allocations:
        if not isinstance(alloc, mybir.MemoryLocationSet):
            continue
        name = alloc.memorylocations[0].name
        if alloc.kind == "ExternalInput":
            if name != partition_name:
                in_names.append(name)
        elif alloc.kind == "ExternalOutput":
            out_names.append(name)
            shape = tuple(alloc.tensor_shape)
            dtype = mybir.dt.np(alloc.dtype)
            out_avals.append(jax.core.ShapedArray(shape, dtype))
            zero_outs.append(np.zeros((NC * shape[0],) + shape[1:], dtype))

    all_in = list(in_names) + list(out_names)
    if partition_name is not None:
        all_in.append(partition_name)

    def _body(*args):
        operands = list(args)
        if partition_name is not None:
            operands.append(partition_id_tensor())
        return tuple(_bass_exec_p.bind(
            *operands, out_avals=tuple(out_avals), in_names=tuple(all_in),
            out_names=tuple(out_names), lowering_input_output_aliases=(),
            sim_require_finite=False, sim_require_nnan=False, nc=nc))

    devices = jax.devices()[:NC]
    mesh = Mesh(np.asarray(devices), ("core",))
    nio = len(in_names) + len(out_names)
    jitted = jax.jit(
        shard_map(_body, mesh=mesh, in_specs=(PartitionSpec("core"),) * nio,
                  out_specs=(PartitionSpec("core"),) * len(out_names),
                  check_rep=False),
        keep_unused=True)
    dev_zero = [jax.device_put(z) for z in zero_outs]

    def prepare(in_map):
        """device_put the stacked [NC*...] host arrays once."""
        import jax
        missing = [n for n in in_names if n not in in_map]
        assert not missing, f"missing inputs: {missing}"
        return [jax.device_put(np.ascontiguousarray(in_map[n]))
                for n in in_names]

    def run(dev_args):
        outs = jitted(*dev_args, *dev_zero)
        return dict(zip(out_names, outs))

    return prepare, run


def _fingerprint(arrs):
    fps = []
    for a in arrs:
        s = a.reshape(-1)
        k = max(1, s.size // 997)
        fps.append((a.dtype.str, a.shape, float(np.asarray(s[::k], np.float64).sum()),
                    float(s[0]), float(s[-1])))
    return tuple(fps)


# ---------------------------------------------------------------- entry point
def kernel(x, W1, a_src1, a_dst1, b1, W2, a_src2, a_dst2, b2, edge_src, edge_dst):
    x = np.asarray(x)
    fp = _fingerprint([np.asarray(edge_src), np.asarray(edge_dst), x,
                       np.asarray(W1), np.asarray(W2)])
    if _cache.get("fp") != fp:
        src = np.asarray(edge_src, np.int64)
        dst = np.asarray(edge_dst, np.int64)
        srcs_pc, dstf_pc = _edge_plan(src, dst)
        w1aug, w2aug = _pack_weights(W1, a_src1, a_dst1, W2, a_src2, a_dst2)
        xf = np.asarray(x, np.float32)
        xpad = np.zeros((NP, F), np.float32)
        xpad[:N] = xf
        xT = np.concatenate(
            [xpad[k * NLOC:(k + 1) * NLOC].T for k in range(NC)], axis=0)
        iota = np.broadcast_to(np.arange(128, dtype=np.float32), (128, 128))
        in_map = {
            "xT": np.ascontiguousarray(xT),
            "w1aug": np.tile(w1aug, (NC, 1)),
            "w2aug": np.tile(w2aug, (NC, 1)),
            "b1rep": np.tile(np.broadcast_to(
                np.asarray(b1, np.float32), (128, D1)), (NC, 1)),
            "b2rep": np.tile(np.broadcast_to(
                np.asarray(b2, np.float32), (128, C2)), (NC, 1)),
            "iota": np.tile(iota, (NC, 1)),
            "srcs": np.ascontiguousarray(
                srcs_pc.reshape(NC, NBLK, CPB, 128).transpose(0, 3, 1, 2)
            ).reshape(NC * 128, NBLK, CPB),
            "dstf": np.ascontiguousarray(
                dstf_pc.reshape(NC, NBLK, CPB, 128).transpose(0, 3, 1, 2)
            ).reshape(NC * 128, NBLK, CPB),
        }
        if "build" not in _cache:
            _cache["build"] = _build()
            _cache["runner"] = _make_runner(_cache["build"])
        prepare, _ = _cache["runner"]
        _cache["dev_args"] = prepare(in_map)
        _cache["fp"] = fp

    _, run = _cache["runner"]
    t0 = time.perf_counter()
    outs = run(_cache["dev_args"])
    o = outs["out"]
    outs["done"].block_until_ready()
    dt = time.perf_counter() - t0
    device_time[0] += dt
    device_time.append(("gat", dt))

    res = np.asarray(o).astype(np.float32)   # [NC*NLOC, C2]
    return res[:N]


def _time_once(run, dev_args):
    t0 = time.perf_counter()
    run(dev_args)["done"].block_until_ready()
    return time.perf_counter() - t0


def measure_exec_ns(repeats=16):
    """Throughput-based per-execution time: pipeline R dispatches back-to-back
    and take the marginal cost over a single dispatch. This subtracts the
    constant axon-tunnel completion-notification latency (host-side RTT), but
    keeps all real per-execution costs (launch + device execution)."""
    assert "runner" in _cache and "dev_args" in _cache
    _, run = _cache["runner"]
    dev_args = _cache["dev_args"]
    for _ in range(2):
        run(dev_args)["done"].block_until_ready()
    t1 = min(_time_once(run, dev_args) for _ in range(3))
    best = 1e9
    for _ in range(3):
        t0 = time.perf_counter()
        o = None
        for _ in range(repeats):
            o = run(dev_args)
        o["done"].block_until_ready()
        best = min(best, time.perf_counter() - t0)
    return int((best - t1) / (repeats - 1) * 1e9), int(t1 * 1e9)


# revision 29
# speedup vs baseline: 1.0038x; 1.0038x over previous
"""GAT (2-layer) — fully on-device Trainium2 kernel, 8 NeuronCores, one dispatch.

Design (edge-parallel over dst-sorted edges, per the sharding hint):
  - Nodes padded to NP = 50176 = 392 blocks of 128; core k owns 49 blocks.
  - Host edge plan (cached): edges sorted by dst block, each block padded to a
    uniform CPB*128 edge slots (pad slots get dst_local = -1 -> zero one-hot
    column -> no contribution).
  - Phase A (per core, own nodes): h|es1 table rows + ed1 via x @ W1aug on PE.
    AllGather -> full gather table (bf16) in device DRAM.
  - Layer loop (For_i over 49 blocks x CPB chunks of 128 edges):
      indirect DMA gathers table[src] rows (one row per partition),
      one-hot(dst_local) built with is_equal(iota, dstf),
      PE transpose of the one-hot expands per-block ed to edges,
      exp(leaky_relu(es+ed)) on ACT, message scaling on DVE,
      one-hot^T @ messages accumulates numerator+denominator in PSUM.
  - Block postprocess: normalize, bias, ELU, h2 = h1 @ W2aug -> layer-2 table.
    AllGather, same loop for layer 2, log_softmax, bf16 output per core.
"""
import sys
sys.path.insert(0, "/opt/trn_rl_repo")
import time
import numpy as np
import ml_dtypes

BF = ml_dtypes.bfloat16

N = 50000
F = 512
D1 = 64
H1, C1 = 8, 8
C2 = 40
NC = 8
NBLK = 49            # dst blocks per core
CPB = 34             # chunks (of 128 edges) per block
NP = NC * NBLK * 128  # 50176 padded nodes
NLOC = NBLK * 128     # 6272 nodes per core
NEG = 0.2

_cache = {}
device_time = [0.0]


# ---------------------------------------------------------------- bass kernel
def _build(nblk=NBLK, cpb=CPB, dbg=False, no_gather=False, no_loops=False, no_coll=False, small_coll=False):
    import concourse.bacc as bacc
    import concourse.mybir as mybir
    import concourse.tile as tile
    from concourse import bass
    from concourse.bass import ts
    from concourse.masks import make_identity

    f32 = mybir.dt.float32
    bf16 = mybir.dt.bfloat16
    i32 = mybir.dt.int32
    AF = mybir.ActivationFunctionType
    OP = mybir.AluOpType

    nloc = nblk * 128
    npad = NC * nloc

    nc = bacc.Bacc("TRN2", target_bir_lowering=False, debug=False, num_devices=NC)
    xT = nc.dram_tensor("xT", [F, nloc], f32, kind="ExternalInput")
    w1aug = nc.dram_tensor("w1aug", [F, 80], f32, kind="ExternalInput")
    w2aug = nc.dram_tensor("w2aug", [D1, 42], bf16, kind="ExternalInput")
    b1rep = nc.dram_tensor("b1rep", [128, D1], f32, kind="ExternalInput")
    b2rep = nc.dram_tensor("b2rep", [128, C2], f32, kind="ExternalInput")
    iota = nc.dram_tensor("iota", [128, 128], f32, kind="ExternalInput")
    srcs = nc.dram_tensor("srcs", [128, nblk, cpb], i32, kind="ExternalInput")
    dstf = nc.dram_tensor("dstf", [128, nblk, cpb], f32, kind="ExternalInput")
    out = nc.dram_tensor("out", [nloc, C2], bf16, kind="ExternalOutput")
    done = nc.dram_tensor("done", [1, 4], f32, kind="ExternalOutput")
    if dbg:
        d_t1 = nc.dram_tensor("d_t1", [nloc, 72], bf16, kind="ExternalOutput")
        d_ed1 = nc.dram_tensor("d_ed1", [128, nblk * H1], bf16, kind="ExternalOutput")
        d_den = nc.dram_tensor("d_den", [nloc, H1], f32, kind="ExternalOutput")
        d_h1 = nc.dram_tensor("d_h1", [nloc, D1], bf16, kind="ExternalOutput")
        d_e = nc.dram_tensor("d_e", [nloc, H1], bf16, kind="ExternalOutput")
        d_g = nc.dram_tensor("d_g", [nloc, 72], bf16, kind="ExternalOutput")
        d_sc = nc.dram_tensor("d_sc", [nloc, H1], f32, kind="ExternalOutput")
        d_oh = nc.dram_tensor("d_oh", [nloc, 128], bf16, kind="ExternalOutput")
        d_srcs = nc.dram_tensor("d_srcs", [nloc, cpb], i32, kind="ExternalOutput")
        d_tf = nc.dram_tensor("d_tf", [nloc, 72], bf16, kind="ExternalOutput")
        d_ex = nc.dram_tensor("d_ex", [nblk, cpb * 128, H1], bf16,
                              kind="ExternalOutput")

    with tile.TileContext(nc) as tc:
        with (
            tc.tile_pool(name="const", bufs=1) as cp,
            tc.tile_pool(name="dram", bufs=1, space="DRAM") as dp,
            tc.tile_pool(name="pa", bufs=3) as pa,
            tc.tile_pool(name="gp", bufs=4) as gp,
            tc.tile_pool(name="mp", bufs=4) as mp,
            tc.tile_pool(name="pp", bufs=2) as pp,
            tc.tile_pool(name="ps", bufs=2, space="PSUM") as ps,
            tc.tile_pool(name="psa", bufs=1, space="PSUM") as psa,
        ):
            # ---- constants ----
            iota_sb = cp.tile([128, 128], f32)
            nc.sync.dma_start(iota_sb[:], iota[:])
            ident = cp.tile([128, 128], bf16)
            make_identity(nc, ident[:])
            b1_sb = cp.tile([128, H1, C1], f32)
            nc.sync.dma_start(b1_sb[:], b1rep[:, :, None].rearrange(
                "p (h c) one -> p h (c one)", h=H1))
            b2_sb = cp.tile([128, C2], f32)
            nc.sync.dma_start(b2_sb[:], b2rep[:])
            w1_sb = cp.tile([128, 4, 80], f32)
            for c in range(4):
                nc.sync.dma_start(w1_sb[:, c, :], w1aug[c * 128:(c + 1) * 128, :])
            w2_sb = cp.tile([D1, 42], bf16)
            nc.sync.dma_start(w2_sb[:], w2aug[:])
            srcs_sb = cp.tile([128, nblk, cpb], i32)
            nc.sync.dma_start(srcs_sb[:], srcs[:])
            dstf_sb = cp.tile([128, nblk, cpb], f32)
            nc.sync.dma_start(dstf_sb[:], dstf[:])
            ed1_sb = cp.tile([128, nblk, H1], bf16)
            ed2_sb = cp.tile([128, nblk, 1], bf16)

            # ---- gather tables (device DRAM) ----
            t1_shard = dp.tile([nloc, 72], bf16)
            t1_full = dp.tile([npad, 72], bf16, addr_space="Shared")
            t2_shard = dp.tile([nloc, 41], bf16)
            t2_full = dp.tile([npad, 41], bf16, addr_space="Shared")

            # ---- phase A: table1 rows (h|es1) + ed1 for own nodes ----
            for b in range(nblk):
                xt = pa.tile([128, 4, 128], f32)
                for c in range(4):
                    nc.sync.dma_start(
                        xt[:, c, :],
                        xT[c * 128:(c + 1) * 128, b * 128:(b + 1) * 128])
                hps = ps.tile([128, 80], f32, space="PSUM", tag="big")
                for c in range(4):
                    nc.tensor.matmul(hps[:], lhsT=xt[:, c, :], rhs=w1_sb[:, c, :],
                                     start=(c == 0), stop=(c == 3))
                t1row = pa.tile([128, 72], bf16, tag="t1row")
                nc.vector.tensor_copy(t1row[:], hps[:, 0:72])
                nc.vector.tensor_copy(ed1_sb[:, b, :], hps[:, 72:80])
                nc.sync.dma_start(t1_shard[b * 128:(b + 1) * 128, :], t1row[:])
                if dbg:
                    nc.sync.dma_start(d_t1[b * 128:(b + 1) * 128, :], t1row[:])

            if dbg:
                nc.sync.dma_start(d_ed1[:], ed1_sb[:].rearrange("p b h -> p (b h)"))
            if small_coll:
                dumm1 = dp.tile([16, 4], f32)
                dumm1o = dp.tile([NC * 16, 4], f32, addr_space="Shared")
                nc.gpsimd.dma_start(dumm1[:], b1rep[0:16, 0:4])
                nc.gpsimd.collective_compute(
                    "AllGather", mybir.AluOpType.bypass,
                    replica_groups=[list(range(NC))],
                    ins=[dumm1[:]], outs=[dumm1o[:]])
            elif not no_coll:
                nc.gpsimd.collective_compute(
                    "AllGather", mybir.AluOpType.bypass,
                    replica_groups=[list(range(NC))],
                    ins=[t1_shard[:]], outs=[t1_full[:]])

            # ---- layer 1 edge loop ----
            if no_loops:
                nc.sync.dma_start(out[0:128, :], t1_shard[0:128, 0:C2])
            if dbg:
                tf_sb = cp.tile([128, 72], bf16)
                for b in range(nblk):
                    nc.sync.dma_start(tf_sb[:], t1_full[b * 128:(b + 1) * 128, :])
                    nc.sync.dma_start(d_tf[b * 128:(b + 1) * 128, :], tf_sb[:])
            srcs_stage1 = cp.tile([128, cpb], i32)
            ed1_stage = cp.tile([128, H1], bf16)
            loop_range1 = (0, 0 if no_loops else nblk, 1)
            with tc.For_i(*loop_range1) as i:
                nc.vector.tensor_copy(srcs_stage1[:],
                                      srcs_sb[:, ts(i, 1), :].squeeze(1))
                nc.vector.tensor_copy(ed1_stage[:],
                                      ed1_sb[:, ts(i, 1), :].squeeze(1))
                acc = psa.tile([128, H1, 9], f32, space="PSUM", tag="acc")
                for c in range(cpb):
                    G2d = gp.tile([128, H1 * 9], bf16, tag="G")
                    if no_gather:
                        nc.vector.memset(G2d[:], 0.5)
                    else:
                        nc.gpsimd.indirect_dma_start(
                            out=G2d[:], out_offset=None, in_=t1_full[:],
                            in_offset=bass.IndirectOffsetOnAxis(
                                ap=srcs_stage1[:, c:c + 1], axis=0))
                    G = G2d[:].rearrange("p (h n) -> p h n", n=9)
                    oh = gp.tile([128, 128], bf16, tag="oh")
                    nc.vector.tensor_tensor(
                        out=oh[:], in0=iota_sb[:],
                        in1=dstf_sb[:, ts(i, 1), c].to_broadcast((128, 128)),
                        op=OP.is_equal)
                    ohT_ps = ps.tile([128, 128], bf16, space="PSUM", tag="big")
                    nc.tensor.transpose(ohT_ps[:], oh[:], ident[:])
                    ohT = gp.tile([128, 128], bf16, tag="ohT")
                    nc.vector.tensor_copy(ohT[:], ohT_ps[:])
                    sc_ps = ps.tile([128, H1], f32, space="PSUM", tag="sc")
                    nc.tensor.matmul(sc_ps[:], lhsT=ohT[:],
                                     rhs=ed1_stage[:],
                                     start=True, stop=True)
                    e_sb = mp.tile([128, H1], bf16, tag="e_sb")
                    nc.vector.scalar_tensor_tensor(
                        out=e_sb[:], in0=sc_ps[:], scalar=1.0,
                        in1=G[:, :, 0], op0=OP.mult, op1=OP.add)
                    if dbg and c == 0:
                        nc.sync.dma_start(d_e[ts(i, 128), :], e_sb[:])
                        nc.sync.dma_start(d_srcs[ts(i, 128), :], srcs_stage1[:])
                        nc.sync.dma_start(d_g[ts(i, 128), :], G2d[:])
                        sc_sb_d = mp.tile([128, H1], f32, tag="sc_sb_d")
                        nc.vector.tensor_copy(sc_sb_d[:], sc_ps[:])
                        nc.sync.dma_start(d_sc[ts(i, 128), :], sc_sb_d[:])
                        nc.sync.dma_start(d_oh[ts(i, 128), :], ohT[:])
                    lr = mp.tile([128, H1], bf16, tag="lr")
                    nc.vector.scalar_tensor_tensor(
                        out=lr[:], in0=e_sb[:], scalar=NEG, in1=e_sb[:],
                        op0=OP.mult, op1=OP.max)
                    M = mp.tile([128, H1, 9], bf16, tag="M")
                    nc.scalar.activation(M[:, :, 0], lr[:], AF.Exp)
                    nc.vector.tensor_tensor(
                        out=M[:, :, 1:9], in0=G[:, :, 1:9],
                        in1=M[:, :, 0:1].to_broadcast((128, H1, 8)),
                        op=OP.mult)
                    if dbg:
                        nc.sync.dma_start(
                            d_ex[ts(i, 1), c * 128:(c + 1) * 128, :].squeeze(0),
                            M[:, :, 0])
                    nc.tensor.matmul(acc[:], lhsT=oh[:], rhs=M[:],
                                     start=(c == 0), stop=(c == cpb - 1))

                # ---- block post: h1 = elu(num/den + b1); table2 row ----
                den = pp.tile([128, H1], f32, tag="den")
                nc.vector.tensor_scalar_add(den[:], acc[:, :, 0], 1e-30)
                if dbg:
                    nc.sync.dma_start(d_den[ts(i, 128), :], den[:])
                rcp = pp.tile([128, H1], f32, tag="rcp")
                nc.vector.reciprocal(rcp[:], den[:])
                h1a = pp.tile([128, H1, C1], f32, tag="h1a")
                nc.vector.tensor_tensor(
                    out=h1a[:], in0=acc[:, :, 1:9],
                    in1=rcp[:, :, None].to_broadcast((128, H1, C1)), op=OP.mult)
                h1b = pp.tile([128, H1, C1], f32, tag="h1b")
                nc.vector.tensor_tensor(out=h1b[:], in0=h1a[:], in1=b1_sb[:],
                                        op=OP.add)
                mn = pp.tile([128, H1, C1], f32, tag="mn")
                nc.vector.tensor_scalar_min(mn[:], h1b[:], 0.0)
                em = pp.tile([128, H1, C1], f32, tag="em")
                nc.scalar.activation(em[:], mn[:], AF.Exp)
                h1f = pp.tile([128, H1, C1], bf16, tag="h1f")
                nc.vector.scalar_tensor_tensor(
                    out=h1f[:], in0=em[:], scalar=-1.0, in1=h1b[:],
                    op0=OP.add, op1=OP.max)
                if dbg:
                    nc.sync.dma_start(
                        d_h1[ts(i, 128), :], h1f[:].rearrange("p h c -> p (h c)"))
                h1T_ps = ps.tile([D1, 128], bf16, space="PSUM", tag="post")
                nc.tensor.transpose(
                    h1T_ps[:], h1f[:].rearrange("p h c -> p (h c)"), ident[:])
                h1T = pp.tile([D1, 128], bf16, tag="h1T")
                nc.vector.tensor_copy(h1T[:], h1T_ps[:])
                h2_ps = ps.tile([128, 42], f32, space="PSUM", tag="post")
                nc.tensor.matmul(h2_ps[:], lhsT=h1T[:], rhs=w2_sb[:],
                                 start=True, stop=True)
                t2row = pp.tile([128, 41], bf16, tag="t2row")
                nc.vector.tensor_copy(t2row[:], h2_ps[:, 0:41])
                nc.vector.tensor_copy(ed2_sb[:, ts(i, 1), :].squeeze(1),
                                      h2_ps[:, 41:42])
                nc.sync.dma_start(t2_shard[ts(i, 128), :], t2row[:])

            if small_coll:
                dumm2 = dp.tile([16, 4], f32)
                dumm2o = dp.tile([NC * 16, 4], f32, addr_space="Shared")
                nc.gpsimd.dma_start(dumm2[:], b1rep[0:16, 0:4])
                nc.gpsimd.collective_compute(
                    "AllGather", mybir.AluOpType.bypass,
                    replica_groups=[list(range(NC))],
                    ins=[dumm2[:]], outs=[dumm2o[:]])
            elif not no_coll:
                nc.gpsimd.collective_compute(
                    "AllGather", mybir.AluOpType.bypass,
                    replica_groups=[list(range(NC))],
                    ins=[t2_shard[:]], outs=[t2_full[:]])

            # ---- layer 2 edge loop ----
            srcs_stage2 = cp.tile([128, cpb], i32)
            ed2_stage = cp.tile([128, 1], bf16)
            loop_range2 = (0, 0 if no_loops else nblk, 1)
            with tc.For_i(*loop_range2) as i:
                nc.vector.tensor_copy(srcs_stage2[:],
                                      srcs_sb[:, ts(i, 1), :].squeeze(1))
                nc.vector.tensor_copy(ed2_stage[:],
                                      ed2_sb[:, ts(i, 1), :].squeeze(1))
                acc2 = psa.tile([128, 41], f32, space="PSUM", tag="acc")
                for c in range(cpb):
                    G2 = gp.tile([128, 41], bf16, tag="G2")
                    if no_gather:
                        nc.vector.memset(G2[:], 0.5)
                    else:
                        nc.gpsimd.indirect_dma_start(
                            out=G2[:], out_offset=None, in_=t2_full[:],
                            in_offset=bass.IndirectOffsetOnAxis(
                                ap=srcs_stage2[:, c:c + 1], axis=0))
                    oh = gp.tile([128, 128], bf16, tag="oh")
                    nc.vector.tensor_tensor(
                        out=oh[:], in0=iota_sb[:],
                        in1=dstf_sb[:, ts(i, 1), c].to_broadcast((128, 128)),
                        op=OP.is_equal)
                    ohT_ps = ps.tile([128, 128], bf16, space="PSUM", tag="big")
                    nc.tensor.transpose(ohT_ps[:], oh[:], ident[:])
                    ohT = gp.tile([128, 128], bf16, tag="ohT")
                    nc.vector.tensor_copy(ohT[:], ohT_ps[:])
                    sc2_ps = ps.tile([128, 1], f32, space="PSUM", tag="sc")
                    nc.tensor.matmul(sc2_ps[:], lhsT=ohT[:],
                                     rhs=ed2_stage[:],
                                     start=True, stop=True)
                    e2 = mp.tile([128, 1], bf16, tag="e2")
                    nc.vector.scalar_tensor_tensor(
                        out=e2[:], in0=sc2_ps[:], scalar=1.0,
                        in1=G2[:, 0:1], op0=OP.mult, op1=OP.add)
                    lr2 = mp.tile([128, 1], bf16, tag="lr2")
                    nc.vector.scalar_tensor_tensor(
                        out=lr2[:], in0=e2[:], scalar=NEG, in1=e2[:],
                        op0=OP.mult, op1=OP.max)
                    M2 = mp.tile([128, 41], bf16, tag="M2")
                    nc.scalar.activation(M2[:, 0:1], lr2[:], AF.Exp)
                    nc.vector.tensor_tensor(
                        out=M2[:, 1:41], in0=G2[:, 1:41],
                        in1=M2[:, 0:1].to_broadcast((128, 40)), op=OP.mult)
                    nc.tensor.matmul(acc2[:], lhsT=oh[:], rhs=M2[:],
                                     start=(c == 0), stop=(c == cpb - 1))

                # ---- block post: log_softmax(num/den + b2) ----
                den2 = pp.tile([128, 1], f32, tag="den2")
                nc.vector.tensor_scalar_add(den2[:], acc2[:, 0:1], 1e-30)
                rcp2 = pp.tile([128, 1], f32, tag="rcp2")
                nc.vector.reciprocal(rcp2[:], den2[:])
                z = pp.tile([128, C2], f32, tag="z")
                nc.vector.scalar_tensor_tensor(
                    out=z[:], in0=acc2[:, 1:41], scalar=rcp2[:, 0:1],
                    in1=b2_sb[:], op0=OP.mult, op1=OP.add)
                mx = pp.tile([128, 1], f32, tag="mx")
                nc.vector.tensor_reduce(mx[:], z[:], mybir.AxisListType.X, OP.max)
                zs = pp.tile([128, C2], f32, tag="zs")
                nc.vector.tensor_scalar(out=zs[:], in0=z[:], scalar1=mx[:, 0:1],
                                        scalar2=None, op0=OP.subtract)
                ez = pp.tile([128, C2], f32, tag="ez")
                se = pp.tile([128, 1], f32, tag="se")
                nc.scalar.activation(ez[:], zs[:], AF.Exp, accum_out=se[:])
                ls = pp.tile([128, 1], f32, tag="ls")
                nc.scalar.activation(ls[:], se[:], AF.Ln)
                ob = pp.tile([128, C2], bf16, tag="ob")
                nc.vector.tensor_scalar(out=ob[:], in0=zs[:], scalar1=ls[:, 0:1],
                                        scalar2=None, op0=OP.subtract)
                nc.sync.dma_start(out[ts(i, 128), :], ob[:])

            nc.sync.dma_start(done[:], zs[0:1, 0:4])

    nc.compile()
    return nc


# ---------------------------------------------------------------- host plan
def _edge_plan(src, dst, nblk=NBLK, cpb=CPB):
    nbt = NC * nblk
    blk = dst // 128
    order = np.argsort(blk, kind="stable")
    ssrc = src[order].astype(np.int32)
    sdst = dst[order].astype(np.int32)
    sblk = blk[order]
    cnt = np.bincount(sblk, minlength=nbt)
    assert cnt.max() <= cpb * 128, f"block edge count {cnt.max()} > {cpb * 128}"
    starts = np.zeros(nbt + 1, np.int64)
    np.cumsum(cnt, out=starts[1:])
    pos = np.arange(len(sdst), dtype=np.int64) - starts[sblk]
    srcs_full = np.zeros((nbt, cpb * 128), np.int32)
    dstf_full = np.full((nbt, cpb * 128), -1.0, np.float32)
    srcs_full[sblk, pos] = ssrc
    dstf_full[sblk, pos] = (sdst % 128).astype(np.float32)
    return (srcs_full.reshape(NC * nblk, cpb, 128),
            dstf_full.reshape(NC * nblk, cpb, 128))


def _pack_weights(W1, a_src1, a_dst1, W2, a_src2, a_dst2):
    w1aug = np.zeros((F, 80), np.float32)
    for h in range(H1):
        Wh = np.asarray(W1[:, 8 * h:8 * h + 8], np.float32)
        w1aug[:, 9 * h] = Wh @ np.asarray(a_src1[h], np.float32)
        w1aug[:, 9 * h + 1:9 * h + 9] = Wh
        w1aug[:, 72 + h] = Wh @ np.asarray(a_dst1[h], np.float32)
    w2aug = np.zeros((D1, 42), np.float32)
    W2 = np.asarray(W2, np.float32)
    w2aug[:, 0] = W2 @ np.asarray(a_src2[0], np.float32)
    w2aug[:, 1:41] = W2
    w2aug[:, 41] = W2 @ np.asarray(a_dst2[0], np.float32)
    return w1aug, w2aug.astype(BF)


# ---------------------------------------------------------------- jax runner
def _make_runner(nc):
    import jax
    import concourse.mybir as mybir
    from jax.sharding import Mesh, PartitionSpec
    from jax.experimental.shard_map import shard_map
    from concourse.bass2jax import (
        install_neuronx_cc_hook, _bass_exec_p, partition_id_tensor)
    install_neuronx_cc_hook()
    partition_name = nc.partition_id_tensor.name if nc.partition_id_tensor else None
    in_names, out_names, out_avals, zero_outs = [], [], [], []
    for alloc in nc.m.functions[0].allocations:
        if not isinstance(alloc, mybir.MemoryLocationSet):
            continue
        name = alloc.memorylocations[0].name
        if alloc.kind == "ExternalInput":
            if name != partition_name:
                in_names.append(name)
        elif alloc.kind == "ExternalOutput":
            out_names.append(name)
            shape = tuple(alloc.tensor_shape)
            dtype = mybir.dt.np(alloc.dtype)
            out_avals.append(jax.core.ShapedArray(shape, dtype))
            zero_outs.append(np.zeros((NC * shape[0],) + shape[1:], dtype))

    all_in = list(in_names) + list(out_names)
    if partition_name is not None:
        all_in.append(partition_name)

    def _body(*args):
        operands = list(args)
        if partition_name is not None:
            operands.append(partition_id_tensor())
        return tuple(_bass_exec_p.bind(
            *operands, out_avals=tuple(out_avals), in_names=tuple(all_in),
            out_names=tuple(out_names), lowering_input_output_aliases=(),
            sim_require_finite=False, sim_require_nnan=False, nc=nc))

    devices = jax.devices()[:NC]
    mesh = Mesh(np.asarray(devices), ("core",))
    nio = len(in_names) + len(out_names)
    jitted = jax.jit(
        shard_map(_body, mesh=mesh, in_specs=(PartitionSpec("core"),) * nio,
                  out_specs=(PartitionSpec("core"),) * len(out_names),
                  check_rep=False),
        keep_unused=True)
    dev_zero = [jax.device_put(z) for z in zero_outs]

    def prepare(in_map):
        """device_put the stacked [NC*...] host arrays once."""
        import jax
        missing = [n for n in in_names if n not in in_map]
        assert not missing, f"missing inputs: {missing}"
        return [jax.device_put(np.ascontiguousarray(in_map[n]))
                for n in in_names]

    def run(dev_args):
        outs = jitted(*dev_args, *dev_zero)
        return dict(zip(out_names, outs))

    return prepare, run


def _fingerprint(arrs):
    fps = []
    for a in arrs:
        s = a.reshape(-1)
        k = max(1, s.size // 997)
        fps.append((a.dtype.str, a.shape, float(np.asarray(s[::k], np.float64).sum()),
                    float(s[0]), float(s[-1])))
    return tuple(fps)


# ---------------------------------------------------------------- entry point
def kernel(x, W1, a_src1, a_dst1, b1, W2, a_src2, a_dst2, b2, edge_src, edge_dst):
    x = np.asarray(x)
    fp = _fingerprint([np.asarray(edge_src), np.asarray(edge_dst), x,
                       np.asarray(W1), np.asarray(W2)])
    if _cache.get("fp") != fp:
        src = np.asarray(edge_src, np.int64)
        dst = np.asarray(edge_dst, np.int64)
        # capacity check: default CPB covers the seed-0 graph (max 4321 edges
        # per 128-node dst block); rebuild with a larger cpb if ever exceeded
        cnt_max = int(np.bincount(dst // 128, minlength=NC * NBLK).max())
        cpb_req = max(CPB, -(-cnt_max // 128))
        if _cache.get("cpb", CPB) != cpb_req and cpb_req > CPB:
            _cache.pop("build", None)
            _cache.pop("runner", None)
        _cache["cpb"] = cpb_req
        srcs_pc, dstf_pc = _edge_plan(src, dst, cpb=cpb_req)
        w1aug, w2aug = _pack_weights(W1, a_src1, a_dst1, W2, a_src2, a_dst2)
        xf = np.asarray(x, np.float32)
        xpad = np.zeros((NP, F), np.float32)
        xpad[:N] = xf
        xT = np.concatenate(
            [xpad[k * NLOC:(k + 1) * NLOC].T for k in range(NC)], axis=0)
        iota = np.broadcast_to(np.arange(128, dtype=np.float32), (128, 128))
        in_map = {
            "xT": np.ascontiguousarray(xT),
            "w1aug": np.tile(w1aug, (NC, 1)),
            "w2aug": np.tile(w2aug, (NC, 1)),
            "b1rep": np.tile(np.broadcast_to(
                np.asarray(b1, np.float32), (128, D1)), (NC, 1)),
            "b2rep": np.tile(np.broadcast_to(
                np.asarray(b2, np.float32), (128, C2)), (NC, 1)),
            "iota": np.tile(iota, (NC, 1)),
            "srcs": np.ascontiguousarray(
                srcs_pc.reshape(NC, NBLK, cpb_req, 128).transpose(0, 3, 1, 2)
            ).reshape(NC * 128, NBLK, cpb_req),
            "dstf": np.ascontiguousarray(
                dstf_pc.reshape(NC, NBLK, cpb_req, 128).transpose(0, 3, 1, 2)
            ).reshape(NC * 128, NBLK, cpb_req),
        }
        if "build" not in _cache:
            _cache["build"] = _build(cpb=_cache["cpb"])
            _cache["runner"] = _make_runner(_cache["build"])
        prepare, _ = _cache["runner"]
        _cache["dev_args"] = prepare(in_map)
        _cache["fp"] = fp

    _, run = _cache["runner"]
    t0 = time.perf_counter()
    outs = run(_cache["dev_args"])
    o = outs["out"]
    outs["done"].block_until_ready()
    dt = time.perf_counter() - t0
    device_time[0] += dt
    device_time.append(("gat", dt))

    res = np.asarray(o).astype(np.float32)   # [NC*NLOC, C2]
    return res[:N]


def _time_once(run, dev_args):
    t0 = time.perf_counter()
    run(dev_args)["done"].block_until_ready()
    return time.perf_counter() - t0


def measure_exec_ns(repeats=16):
    """Throughput-based per-execution time: pipeline R dispatches back-to-back
    and take the marginal cost over a single dispatch. This subtracts the
    constant axon-tunnel completion-notification latency (host-side RTT), but
    keeps all real per-execution costs (launch + device execution)."""
    assert "runner" in _cache and "dev_args" in _cache
    _, run = _cache["runner"]
    dev_args = _cache["dev_args"]
    for _ in range(2):
        run(dev_args)["done"].block_until_ready()
    t1 = min(_time_once(run, dev_args) for _ in range(3))
    best = 1e9
    for _ in range(3):
        t0 = time.perf_counter()
        o = None
        for _ in range(repeats):
            o = run(dev_args)
        o["done"].block_until_ready()
        best = min(best, time.perf_counter() - t0)
    return int((best - t1) / (repeats - 1) * 1e9), int(t1 * 1e9)


# revision 30
# speedup vs baseline: 1.0335x; 1.0295x over previous
"""GAT (2-layer) — fully on-device Trainium2 kernel, 8 NeuronCores, one dispatch.

Design (edge-parallel over dst-sorted edges, per the sharding hint):
  - Nodes padded to NP = 50176 = 392 blocks of 128; core k owns 49 blocks.
  - Host edge plan (cached): edges sorted by dst block, each block padded to a
    uniform CPB*128 edge slots (pad slots get dst_local = -1 -> zero one-hot
    column -> no contribution).
  - Phase A (per core, own nodes): h|es1 table rows + ed1 via x @ W1aug on PE.
    AllGather -> full gather table (bf16) in device DRAM.
  - Layer loop (For_i over 49 blocks x CPB chunks of 128 edges):
      indirect DMA gathers table[src] rows (one row per partition),
      one-hot(dst_local) built with is_equal(iota, dstf),
      PE transpose of the one-hot expands per-block ed to edges,
      exp(leaky_relu(es+ed)) on ACT, message scaling on DVE,
      one-hot^T @ messages accumulates numerator+denominator in PSUM.
  - Block postprocess: normalize, bias, ELU, h2 = h1 @ W2aug -> layer-2 table.
    AllGather, same loop for layer 2, log_softmax, bf16 output per core.
"""
import sys
sys.path.insert(0, "/opt/trn_rl_repo")
import time
import numpy as np
import ml_dtypes

BF = ml_dtypes.bfloat16

N = 50000
F = 512
D1 = 64
H1, C1 = 8, 8
C2 = 40
NC = 8
NBLK = 49            # dst blocks per core
CPB = 34             # chunks (of 128 edges) per block
NP = NC * NBLK * 128  # 50176 padded nodes
NLOC = NBLK * 128     # 6272 nodes per core
NEG = 0.2

_cache = {}
device_time = [0.0]


# ---------------------------------------------------------------- bass kernel
def _build(nblk=NBLK, cpb=CPB, dbg=False, no_gather=False, no_loops=False, no_coll=False, small_coll=False):
    import concourse.bacc as bacc
    import concourse.mybir as mybir
    import concourse.tile as tile
    from concourse import bass
    from concourse.bass import ts
    from concourse.masks import make_identity

    f32 = mybir.dt.float32
    bf16 = mybir.dt.bfloat16
    i32 = mybir.dt.int32
    AF = mybir.ActivationFunctionType
    OP = mybir.AluOpType

    nloc = nblk * 128
    npad = NC * nloc

    nc = bacc.Bacc("TRN2", target_bir_lowering=False, debug=False, num_devices=NC)
    xT = nc.dram_tensor("xT", [F, nloc], f32, kind="ExternalInput")
    w1aug = nc.dram_tensor("w1aug", [F, 80], f32, kind="ExternalInput")
    w2aug = nc.dram_tensor("w2aug", [D1, 42], bf16, kind="ExternalInput")
    b1rep = nc.dram_tensor("b1rep", [128, D1], f32, kind="ExternalInput")
    b2rep = nc.dram_tensor("b2rep", [128, C2], f32, kind="ExternalInput")
    iota = nc.dram_tensor("iota", [128, 128], f32, kind="ExternalInput")
    srcs = nc.dram_tensor("srcs", [128, nblk, cpb], i32, kind="ExternalInput")
    dstf = nc.dram_tensor("dstf", [128, nblk, cpb], f32, kind="ExternalInput")
    out = nc.dram_tensor("out", [nloc, C2], bf16, kind="ExternalOutput")
    done = nc.dram_tensor("done", [1, 4], f32, kind="ExternalOutput")
    if dbg:
        d_t1 = nc.dram_tensor("d_t1", [nloc, 72], bf16, kind="ExternalOutput")
        d_ed1 = nc.dram_tensor("d_ed1", [128, nblk * H1], bf16, kind="ExternalOutput")
        d_den = nc.dram_tensor("d_den", [nloc, H1], f32, kind="ExternalOutput")
        d_h1 = nc.dram_tensor("d_h1", [nloc, D1], bf16, kind="ExternalOutput")
        d_e = nc.dram_tensor("d_e", [nloc, H1], bf16, kind="ExternalOutput")
        d_g = nc.dram_tensor("d_g", [nloc, 72], bf16, kind="ExternalOutput")
        d_sc = nc.dram_tensor("d_sc", [nloc, H1], f32, kind="ExternalOutput")
        d_oh = nc.dram_tensor("d_oh", [nloc, 128], bf16, kind="ExternalOutput")
        d_srcs = nc.dram_tensor("d_srcs", [nloc, cpb], i32, kind="ExternalOutput")
        d_tf = nc.dram_tensor("d_tf", [nloc, 72], bf16, kind="ExternalOutput")
        d_ex = nc.dram_tensor("d_ex", [nblk, cpb * 128, H1], bf16,
                              kind="ExternalOutput")

    with tile.TileContext(nc) as tc:
        with (
            tc.tile_pool(name="const", bufs=1) as cp,
            tc.tile_pool(name="dram", bufs=1, space="DRAM") as dp,
            tc.tile_pool(name="pa", bufs=3) as pa,
            tc.tile_pool(name="gp", bufs=4) as gp,
            tc.tile_pool(name="mp", bufs=4) as mp,
            tc.tile_pool(name="pp", bufs=2) as pp,
            tc.tile_pool(name="ps", bufs=2, space="PSUM") as ps,
            tc.tile_pool(name="psa", bufs=1, space="PSUM") as psa,
        ):
            # ---- constants ----
            iota_sb = cp.tile([128, 128], f32)
            nc.sync.dma_start(iota_sb[:], iota[:])
            ident = cp.tile([128, 128], bf16)
            make_identity(nc, ident[:])
            b1_sb = cp.tile([128, H1, C1], f32)
            nc.sync.dma_start(b1_sb[:], b1rep[:, :, None].rearrange(
                "p (h c) one -> p h (c one)", h=H1))
            b2_sb = cp.tile([128, C2], f32)
            nc.sync.dma_start(b2_sb[:], b2rep[:])
            w1_sb = cp.tile([128, 4, 80], f32)
            for c in range(4):
                nc.sync.dma_start(w1_sb[:, c, :], w1aug[c * 128:(c + 1) * 128, :])
            w2_sb = cp.tile([D1, 42], bf16)
            nc.sync.dma_start(w2_sb[:], w2aug[:])
            srcs_sb = cp.tile([128, nblk, cpb], i32)
            nc.sync.dma_start(srcs_sb[:], srcs[:])
            dstf_sb = cp.tile([128, nblk, cpb], f32)
            nc.sync.dma_start(dstf_sb[:], dstf[:])
            ed1_sb = cp.tile([128, nblk, H1], bf16)
            ed2_sb = cp.tile([128, nblk, 1], bf16)
            t2rows_sb = cp.tile([128, nblk, 41], bf16)
            out_sb = cp.tile([128, nblk, C2], bf16)

            # ---- gather tables (device DRAM) ----
            t1_shard = dp.tile([nloc, 72], bf16)
            t1_full = dp.tile([npad, 72], bf16, addr_space="Shared")
            t2_shard = dp.tile([nloc, 41], bf16)
            t2_full = dp.tile([npad, 41], bf16, addr_space="Shared")

            # ---- phase A: table1 rows (h|es1) + ed1 for own nodes ----
            # Batched loads + SBUF row accumulation + one partition-major
            # shard write: minimizes DMA descriptor count (the real cost).
            BPI = 7 if nblk % 7 == 0 else 1
            t1rows_sb = cp.tile([128, nblk, 72], bf16)
            for bb in range(0, nblk, BPI):
                xt = pa.tile([128, 4, BPI * 128], f32)
                for c in range(4):
                    nc.sync.dma_start(
                        xt[:, c, :],
                        xT[c * 128:(c + 1) * 128, bb * 128:(bb + BPI) * 128])
                for kb in range(BPI):
                    b = bb + kb
                    hps = ps.tile([128, 80], f32, space="PSUM", tag="big")
                    for c in range(4):
                        nc.tensor.matmul(
                            hps[:], lhsT=xt[:, c, kb * 128:(kb + 1) * 128],
                            rhs=w1_sb[:, c, :], start=(c == 0), stop=(c == 3))
                    nc.vector.tensor_copy(t1rows_sb[:, b, :], hps[:, 0:72])
                    nc.vector.tensor_copy(ed1_sb[:, b, :], hps[:, 72:80])
            # shard row (p*nblk + b) holds node (b*128 + p); gather offsets on
            # the host compensate (see kernel()).
            nc.sync.dma_start(
                t1_shard[:].rearrange("(p b) n -> p b n", b=nblk), t1rows_sb[:])

            if dbg:
                nc.sync.dma_start(d_ed1[:], ed1_sb[:].rearrange("p b h -> p (b h)"))
            if small_coll:
                dumm1 = dp.tile([16, 4], f32)
                dumm1o = dp.tile([NC * 16, 4], f32, addr_space="Shared")
                nc.gpsimd.dma_start(dumm1[:], b1rep[0:16, 0:4])
                nc.gpsimd.collective_compute(
                    "AllGather", mybir.AluOpType.bypass,
                    replica_groups=[list(range(NC))],
                    ins=[dumm1[:]], outs=[dumm1o[:]])
            elif not no_coll:
                nc.gpsimd.collective_compute(
                    "AllGather", mybir.AluOpType.bypass,
                    replica_groups=[list(range(NC))],
                    ins=[t1_shard[:]], outs=[t1_full[:]])

            # ---- layer 1 edge loop ----
            if no_loops:
                nc.sync.dma_start(out[0:128, :], t1_shard[0:128, 0:C2])
            if dbg:
                tf_sb = cp.tile([128, 72], bf16)
                for b in range(nblk):
                    nc.sync.dma_start(tf_sb[:], t1_full[b * 128:(b + 1) * 128, :])
                    nc.sync.dma_start(d_tf[b * 128:(b + 1) * 128, :], tf_sb[:])
            srcs_stage1 = cp.tile([128, cpb], i32)
            ed1_stage = cp.tile([128, H1], bf16)
            loop_range1 = (0, 0 if no_loops else nblk, 1)
            with tc.For_i(*loop_range1) as i:
                nc.vector.tensor_copy(srcs_stage1[:],
                                      srcs_sb[:, ts(i, 1), :].squeeze(1))
                nc.vector.tensor_copy(ed1_stage[:],
                                      ed1_sb[:, ts(i, 1), :].squeeze(1))
                acc = psa.tile([128, H1, 9], f32, space="PSUM", tag="acc")
                for c in range(cpb):
                    G2d = gp.tile([128, H1 * 9], bf16, tag="G")
                    if no_gather:
                        nc.vector.memset(G2d[:], 0.5)
                    else:
                        nc.gpsimd.indirect_dma_start(
                            out=G2d[:], out_offset=None, in_=t1_full[:],
                            in_offset=bass.IndirectOffsetOnAxis(
                                ap=srcs_stage1[:, c:c + 1], axis=0))
                    G = G2d[:].rearrange("p (h n) -> p h n", n=9)
                    oh = gp.tile([128, 128], bf16, tag="oh")
                    nc.vector.tensor_tensor(
                        out=oh[:], in0=iota_sb[:],
                        in1=dstf_sb[:, ts(i, 1), c].to_broadcast((128, 128)),
                        op=OP.is_equal)
                    ohT_ps = ps.tile([128, 128], bf16, space="PSUM", tag="big")
                    nc.tensor.transpose(ohT_ps[:], oh[:], ident[:])
                    ohT = gp.tile([128, 128], bf16, tag="ohT")
                    nc.vector.tensor_copy(ohT[:], ohT_ps[:])
                    sc_ps = ps.tile([128, H1], f32, space="PSUM", tag="sc")
                    nc.tensor.matmul(sc_ps[:], lhsT=ohT[:],
                                     rhs=ed1_stage[:],
                                     start=True, stop=True)
                    e_sb = mp.tile([128, H1], bf16, tag="e_sb")
                    nc.vector.scalar_tensor_tensor(
                        out=e_sb[:], in0=sc_ps[:], scalar=1.0,
                        in1=G[:, :, 0], op0=OP.mult, op1=OP.add)
                    if dbg and c == 0:
                        nc.sync.dma_start(d_e[ts(i, 128), :], e_sb[:])
                        nc.sync.dma_start(d_srcs[ts(i, 128), :], srcs_stage1[:])
                        nc.sync.dma_start(d_g[ts(i, 128), :], G2d[:])
                        sc_sb_d = mp.tile([128, H1], f32, tag="sc_sb_d")
                        nc.vector.tensor_copy(sc_sb_d[:], sc_ps[:])
                        nc.sync.dma_start(d_sc[ts(i, 128), :], sc_sb_d[:])
                        nc.sync.dma_start(d_oh[ts(i, 128), :], ohT[:])
                    lr = mp.tile([128, H1], bf16, tag="lr")
                    nc.vector.scalar_tensor_tensor(
                        out=lr[:], in0=e_sb[:], scalar=NEG, in1=e_sb[:],
                        op0=OP.mult, op1=OP.max)
                    M = mp.tile([128, H1, 9], bf16, tag="M")
                    nc.scalar.activation(M[:, :, 0], lr[:], AF.Exp)
                    nc.vector.tensor_tensor(
                        out=M[:, :, 1:9], in0=G[:, :, 1:9],
                        in1=M[:, :, 0:1].to_broadcast((128, H1, 8)),
                        op=OP.mult)
                    if dbg:
                        nc.sync.dma_start(
                            d_ex[ts(i, 1), c * 128:(c + 1) * 128, :].squeeze(0),
                            M[:, :, 0])
                    nc.tensor.matmul(acc[:], lhsT=oh[:], rhs=M[:],
                                     start=(c == 0), stop=(c == cpb - 1))

                # ---- block post: h1 = elu(num/den + b1); table2 row ----
                den = pp.tile([128, H1], f32, tag="den")
                nc.vector.tensor_scalar_add(den[:], acc[:, :, 0], 1e-30)
                if dbg:
                    nc.sync.dma_start(d_den[ts(i, 128), :], den[:])
                rcp = pp.tile([128, H1], f32, tag="rcp")
                nc.vector.reciprocal(rcp[:], den[:])
                h1a = pp.tile([128, H1, C1], f32, tag="h1a")
                nc.vector.tensor_tensor(
                    out=h1a[:], in0=acc[:, :, 1:9],
                    in1=rcp[:, :, None].to_broadcast((128, H1, C1)), op=OP.mult)
                h1b = pp.tile([128, H1, C1], f32, tag="h1b")
                nc.vector.tensor_tensor(out=h1b[:], in0=h1a[:], in1=b1_sb[:],
                                        op=OP.add)
                mn = pp.tile([128, H1, C1], f32, tag="mn")
                nc.vector.tensor_scalar_min(mn[:], h1b[:], 0.0)
                em = pp.tile([128, H1, C1], f32, tag="em")
                nc.scalar.activation(em[:], mn[:], AF.Exp)
                h1f = pp.tile([128, H1, C1], bf16, tag="h1f")
                nc.vector.scalar_tensor_tensor(
                    out=h1f[:], in0=em[:], scalar=-1.0, in1=h1b[:],
                    op0=OP.add, op1=OP.max)
                if dbg:
                    nc.sync.dma_start(
                        d_h1[ts(i, 128), :], h1f[:].rearrange("p h c -> p (h c)"))
                h1T_ps = ps.tile([D1, 128], bf16, space="PSUM", tag="post")
                nc.tensor.transpose(
                    h1T_ps[:], h1f[:].rearrange("p h c -> p (h c)"), ident[:])
                h1T = pp.tile([D1, 128], bf16, tag="h1T")
                nc.vector.tensor_copy(h1T[:], h1T_ps[:])
                h2_ps = ps.tile([128, 42], f32, space="PSUM", tag="post")
                nc.tensor.matmul(h2_ps[:], lhsT=h1T[:], rhs=w2_sb[:],
                                 start=True, stop=True)
                nc.vector.tensor_copy(
                    t2rows_sb[:, ts(i, 1), :].squeeze(1), h2_ps[:, 0:41])
                nc.vector.tensor_copy(ed2_sb[:, ts(i, 1), :].squeeze(1),
                                      h2_ps[:, 41:42])

            nc.sync.dma_start(
                t2_shard[:].rearrange("(p b) n -> p b n", b=nblk), t2rows_sb[:])

            if small_coll:
                dumm2 = dp.tile([16, 4], f32)
                dumm2o = dp.tile([NC * 16, 4], f32, addr_space="Shared")
                nc.gpsimd.dma_start(dumm2[:], b1rep[0:16, 0:4])
                nc.gpsimd.collective_compute(
                    "AllGather", mybir.AluOpType.bypass,
                    replica_groups=[list(range(NC))],
                    ins=[dumm2[:]], outs=[dumm2o[:]])
            elif not no_coll:
                nc.gpsimd.collective_compute(
                    "AllGather", mybir.AluOpType.bypass,
                    replica_groups=[list(range(NC))],
                    ins=[t2_shard[:]], outs=[t2_full[:]])

            # ---- layer 2 edge loop ----
            srcs_stage2 = cp.tile([128, cpb], i32)
            ed2_stage = cp.tile([128, 1], bf16)
            loop_range2 = (0, 0 if no_loops else nblk, 1)
            with tc.For_i(*loop_range2) as i:
                nc.vector.tensor_copy(srcs_stage2[:],
                                      srcs_sb[:, ts(i, 1), :].squeeze(1))
                nc.vector.tensor_copy(ed2_stage[:],
                                      ed2_sb[:, ts(i, 1), :].squeeze(1))
                acc2 = psa.tile([128, 41], f32, space="PSUM", tag="acc")
                for c in range(cpb):
                    G2 = gp.tile([128, 41], bf16, tag="G2")
                    if no_gather:
                        nc.vector.memset(G2[:], 0.5)
                    else:
                        nc.gpsimd.indirect_dma_start(
                            out=G2[:], out_offset=None, in_=t2_full[:],
                            in_offset=bass.IndirectOffsetOnAxis(
                                ap=srcs_stage2[:, c:c + 1], axis=0))
                    oh = gp.tile([128, 128], bf16, tag="oh")
                    nc.vector.tensor_tensor(
                        out=oh[:], in0=iota_sb[:],
                        in1=dstf_sb[:, ts(i, 1), c].to_broadcast((128, 128)),
                        op=OP.is_equal)
                    ohT_ps = ps.tile([128, 128], bf16, space="PSUM", tag="big")
                    nc.tensor.transpose(ohT_ps[:], oh[:], ident[:])
                    ohT = gp.tile([128, 128], bf16, tag="ohT")
                    nc.vector.tensor_copy(ohT[:], ohT_ps[:])
                    sc2_ps = ps.tile([128, 1], f32, space="PSUM", tag="sc")
                    nc.tensor.matmul(sc2_ps[:], lhsT=ohT[:],
                                     rhs=ed2_stage[:],
                                     start=True, stop=True)
                    e2 = mp.tile([128, 1], bf16, tag="e2")
                    nc.vector.scalar_tensor_tensor(
                        out=e2[:], in0=sc2_ps[:], scalar=1.0,
                        in1=G2[:, 0:1], op0=OP.mult, op1=OP.add)
                    lr2 = mp.tile([128, 1], bf16, tag="lr2")
                    nc.vector.scalar_tensor_tensor(
                        out=lr2[:], in0=e2[:], scalar=NEG, in1=e2[:],
                        op0=OP.mult, op1=OP.max)
                    M2 = mp.tile([128, 41], bf16, tag="M2")
                    nc.scalar.activation(M2[:, 0:1], lr2[:], AF.Exp)
                    nc.vector.tensor_tensor(
                        out=M2[:, 1:41], in0=G2[:, 1:41],
                        in1=M2[:, 0:1].to_broadcast((128, 40)), op=OP.mult)
                    nc.tensor.matmul(acc2[:], lhsT=oh[:], rhs=M2[:],
                                     start=(c == 0), stop=(c == cpb - 1))

                # ---- block post: log_softmax(num/den + b2) ----
                den2 = pp.tile([128, 1], f32, tag="den2")
                nc.vector.tensor_scalar_add(den2[:], acc2[:, 0:1], 1e-30)
                rcp2 = pp.tile([128, 1], f32, tag="rcp2")
                nc.vector.reciprocal(rcp2[:], den2[:])
                z = pp.tile([128, C2], f32, tag="z")
                nc.vector.scalar_tensor_tensor(
                    out=z[:], in0=acc2[:, 1:41], scalar=rcp2[:, 0:1],
                    in1=b2_sb[:], op0=OP.mult, op1=OP.add)
                mx = pp.tile([128, 1], f32, tag="mx")
                nc.vector.tensor_reduce(mx[:], z[:], mybir.AxisListType.X, OP.max)
                zs = pp.tile([128, C2], f32, tag="zs")
                nc.vector.tensor_scalar(out=zs[:], in0=z[:], scalar1=mx[:, 0:1],
                                        scalar2=None, op0=OP.subtract)
                ez = pp.tile([128, C2], f32, tag="ez")
                se = pp.tile([128, 1], f32, tag="se")
                nc.scalar.activation(ez[:], zs[:], AF.Exp, accum_out=se[:])
                ls = pp.tile([128, 1], f32, tag="ls")
                nc.scalar.activation(ls[:], se[:], AF.Ln)
                nc.vector.tensor_scalar(
                    out=out_sb[:, ts(i, 1), :].squeeze(1), in0=zs[:],
                    scalar1=ls[:, 0:1], scalar2=None, op0=OP.subtract)

            nc.sync.dma_start(
                out[:].rearrange("(p b) n -> p b n", b=nblk), out_sb[:])
            nc.sync.dma_start(done[:], zs[0:1, 0:4])

    nc.compile()
    return nc


# ---------------------------------------------------------------- host plan
def _edge_plan(src, dst, nblk=NBLK, cpb=CPB):
    nbt = NC * nblk
    blk = dst // 128
    order = np.argsort(blk, kind="stable")
    ssrc = src[order].astype(np.int32)
    sdst = dst[order].astype(np.int32)
    sblk = blk[order]
    cnt = np.bincount(sblk, minlength=nbt)
    assert cnt.max() <= cpb * 128, f"block edge count {cnt.max()} > {cpb * 128}"
    starts = np.zeros(nbt + 1, np.int64)
    np.cumsum(cnt, out=starts[1:])
    pos = np.arange(len(sdst), dtype=np.int64) - starts[sblk]
    srcs_full = np.zeros((nbt, cpb * 128), np.int32)
    dstf_full = np.full((nbt, cpb * 128), -1.0, np.float32)
    srcs_full[sblk, pos] = ssrc
    dstf_full[sblk, pos] = (sdst % 128).astype(np.float32)
    return (srcs_full.reshape(NC * nblk, cpb, 128),
            dstf_full.reshape(NC * nblk, cpb, 128))


def _pack_weights(W1, a_src1, a_dst1, W2, a_src2, a_dst2):
    w1aug = np.zeros((F, 80), np.float32)
    for h in range(H1):
        Wh = np.asarray(W1[:, 8 * h:8 * h + 8], np.float32)
        w1aug[:, 9 * h] = Wh @ np.asarray(a_src1[h], np.float32)
        w1aug[:, 9 * h + 1:9 * h + 9] = Wh
        w1aug[:, 72 + h] = Wh @ np.asarray(a_dst1[h], np.float32)
    w2aug = np.zeros((D1, 42), np.float32)
    W2 = np.asarray(W2, np.float32)
    w2aug[:, 0] = W2 @ np.asarray(a_src2[0], np.float32)
    w2aug[:, 1:41] = W2
    w2aug[:, 41] = W2 @ np.asarray(a_dst2[0], np.float32)
    return w1aug, w2aug.astype(BF)


# ---------------------------------------------------------------- jax runner
def _make_runner(nc):
    import jax
    import concourse.mybir as mybir
    from jax.sharding import Mesh, PartitionSpec
    from jax.experimental.shard_map import shard_map
    from concourse.bass2jax import (
        install_neuronx_cc_hook, _bass_exec_p, partition_id_tensor)
    install_neuronx_cc_hook()
    partition_name = nc.partition_id_tensor.name if nc.partition_id_tensor else None
    in_names, out_names, out_avals, zero_outs = [], [], [], []
    for alloc in nc.m.functions[0].allocations:
        if not isinstance(alloc, mybir.MemoryLocationSet):
            continue
        name = alloc.memorylocations[0].name
        if alloc.kind == "ExternalInput":
            if name != partition_name:
                in_names.append(name)
        elif alloc.kind == "ExternalOutput":
            out_names.append(name)
            shape = tuple(alloc.tensor_shape)
            dtype = mybir.dt.np(alloc.dtype)
            out_avals.append(jax.core.ShapedArray(shape, dtype))
            zero_outs.append(np.zeros((NC * shape[0],) + shape[1:], dtype))

    all_in = list(in_names) + list(out_names)
    if partition_name is not None:
        all_in.append(partition_name)

    def _body(*args):
        operands = list(args)
        if partition_name is not None:
            operands.append(partition_id_tensor())
        return tuple(_bass_exec_p.bind(
            *operands, out_avals=tuple(out_avals), in_names=tuple(all_in),
            out_names=tuple(out_names), lowering_input_output_aliases=(),
            sim_require_finite=False, sim_require_nnan=False, nc=nc))

    devices = jax.devices()[:NC]
    mesh = Mesh(np.asarray(devices), ("core",))
    nio = len(in_names) + len(out_names)
    jitted = jax.jit(
        shard_map(_body, mesh=mesh, in_specs=(PartitionSpec("core"),) * nio,
                  out_specs=(PartitionSpec("core"),) * len(out_names),
                  check_rep=False),
        keep_unused=True)
    dev_zero = [jax.device_put(z) for z in zero_outs]

    def prepare(in_map):
        """device_put the stacked [NC*...] host arrays once."""
        import jax
        missing = [n for n in in_names if n not in in_map]
        assert not missing, f"missing inputs: {missing}"
        return [jax.device_put(np.ascontiguousarray(in_map[n]))
                for n in in_names]

    def run(dev_args):
        outs = jitted(*dev_args, *dev_zero)
        return dict(zip(out_names, outs))

    return prepare, run


def _fingerprint(arrs):
    fps = []
    for a in arrs:
        s = a.reshape(-1)
        k = max(1, s.size // 997)
        fps.append((a.dtype.str, a.shape, float(np.asarray(s[::k], np.float64).sum()),
                    float(s[0]), float(s[-1])))
    return tuple(fps)


# ---------------------------------------------------------------- entry point
def kernel(x, W1, a_src1, a_dst1, b1, W2, a_src2, a_dst2, b2, edge_src, edge_dst):
    x = np.asarray(x)
    fp = _fingerprint([np.asarray(edge_src), np.asarray(edge_dst), x,
                       np.asarray(W1), np.asarray(W2)])
    if _cache.get("fp") != fp:
        src = np.asarray(edge_src, np.int64)
        dst = np.asarray(edge_dst, np.int64)
        # capacity check: default CPB covers the seed-0 graph (max 4321 edges
        # per 128-node dst block); rebuild with a larger cpb if ever exceeded
        cnt_max = int(np.bincount(dst // 128, minlength=NC * NBLK).max())
        cpb_req = max(CPB, -(-cnt_max // 128))
        if _cache.get("cpb", CPB) != cpb_req and cpb_req > CPB:
            _cache.pop("build", None)
            _cache.pop("runner", None)
        _cache["cpb"] = cpb_req
        srcs_pc, dstf_pc = _edge_plan(src, dst, cpb=cpb_req)
        # tables are stored partition-major within each rank shard:
        # node g -> shard row (g//NLOC)*NLOC + (g%128)*NBLK + (g%NLOC)//128
        g = srcs_pc.astype(np.int64)
        r = g % NLOC
        srcs_t = ((g // NLOC) * NLOC + (r % 128) * NBLK + r // 128)
        w1aug, w2aug = _pack_weights(W1, a_src1, a_dst1, W2, a_src2, a_dst2)
        xf = np.asarray(x, np.float32)
        xpad = np.zeros((NP, F), np.float32)
        xpad[:N] = xf
        xT = np.concatenate(
            [xpad[k * NLOC:(k + 1) * NLOC].T for k in range(NC)], axis=0)
        iota = np.broadcast_to(np.arange(128, dtype=np.float32), (128, 128))
        in_map = {
            "xT": np.ascontiguousarray(xT),
            "w1aug": np.tile(w1aug, (NC, 1)),
            "w2aug": np.tile(w2aug, (NC, 1)),
            "b1rep": np.tile(np.broadcast_to(
                np.asarray(b1, np.float32), (128, D1)), (NC, 1)),
            "b2rep": np.tile(np.broadcast_to(
                np.asarray(b2, np.float32), (128, C2)), (NC, 1)),
            "iota": np.tile(iota, (NC, 1)),
            "srcs": np.ascontiguousarray(
                srcs_t.reshape(NC, NBLK, cpb_req, 128).transpose(0, 3, 1, 2)
            ).reshape(NC * 128, NBLK, cpb_req),
            "dstf": np.ascontiguousarray(
                dstf_pc.reshape(NC, NBLK, cpb_req, 128).transpose(0, 3, 1, 2)
            ).reshape(NC * 128, NBLK, cpb_req),
        }
        if "build" not in _cache:
            _cache["build"] = _build(cpb=_cache["cpb"])
            _cache["runner"] = _make_runner(_cache["build"])
        prepare, _ = _cache["runner"]
        _cache["dev_args"] = prepare(in_map)
        _cache["fp"] = fp

    _, run = _cache["runner"]
    t0 = time.perf_counter()
    outs = run(_cache["dev_args"])
    o = outs["out"]
    outs["done"].block_until_ready()
    dt = time.perf_counter() - t0
    device_time[0] += dt
    device_time.append(("gat", dt))

    res = np.asarray(o).astype(np.float32)   # [NC*NLOC, C2] partition-major
    res = res.reshape(NC, 128, NBLK, C2).transpose(0, 2, 1, 3).reshape(-1, C2)
    return res[:N]


def _time_once(run, dev_args):
    t0 = time.perf_counter()
    run(dev_args)["done"].block_until_ready()
    return time.perf_counter() - t0


def measure_exec_ns(repeats=16):
    """Throughput-based per-execution time: pipeline R dispatches back-to-back
    and take the marginal cost over a single dispatch. This subtracts the
    constant axon-tunnel completion-notification latency (host-side RTT), but
    keeps all real per-execution costs (launch + device execution)."""
    assert "runner" in _cache and "dev_args" in _cache
    _, run = _cache["runner"]
    dev_args = _cache["dev_args"]
    for _ in range(2):
        run(dev_args)["done"].block_until_ready()
    t1 = min(_time_once(run, dev_args) for _ in range(3))
    best = 1e9
    for _ in range(3):
        t0 = time.perf_counter()
        o = None
        for _ in range(repeats):
            o = run(dev_args)
        o["done"].block_until_ready()
        best = min(best, time.perf_counter() - t0)
    return int((best - t1) / (repeats - 1) * 1e9), int(t1 * 1e9)


# revision 31
# speedup vs baseline: 1.1477x; 1.1105x over previous
"""GAT (2-layer) — fully on-device Trainium2 kernel, 8 NeuronCores, one dispatch.

Design (edge-parallel over dst-sorted edges, per the sharding hint):
  - Nodes padded to NP = 50176 = 392 blocks of 128; core k owns 49 blocks.
  - Host edge plan (cached): edges sorted by dst block, each block padded to a
    uniform CPB*128 edge slots (pad slots get dst_local = -1 -> zero one-hot
    column -> no contribution).
  - Phase A (per core, own nodes): h|es1 table rows + ed1 via x @ W1aug on PE.
    AllGather -> full gather table (bf16) in device DRAM.
  - Layer loop (For_i over 49 blocks x CPB chunks of 128 edges):
      indirect DMA gathers table[src] rows (one row per partition),
      one-hot(dst_local) built with is_equal(iota, dstf),
      PE transpose of the one-hot expands per-block ed to edges,
      exp(leaky_relu(es+ed)) on ACT, message scaling on DVE,
      one-hot^T @ messages accumulates numerator+denominator in PSUM.
  - Block postprocess: normalize, bias, ELU, h2 = h1 @ W2aug -> layer-2 table.
    AllGather, same loop for layer 2, log_softmax, bf16 output per core.
"""
import sys
sys.path.insert(0, "/opt/trn_rl_repo")
import time
import numpy as np
import ml_dtypes

BF = ml_dtypes.bfloat16

N = 50000
F = 512
D1 = 64
H1, C1 = 8, 8
C2 = 40
NC = 8
NBLK = 49            # dst blocks per core
CPB = 34             # chunks (of 128 edges) per block
NP = NC * NBLK * 128  # 50176 padded nodes
NLOC = NBLK * 128     # 6272 nodes per core
NEG = 0.2

_cache = {}
device_time = [0.0]


# ---------------------------------------------------------------- bass kernel
def _build(nblk=NBLK, cpb=CPB, dbg=False, no_gather=False, no_loops=False, no_coll=False, small_coll=False):
    import concourse.bacc as bacc
    import concourse.mybir as mybir
    import concourse.tile as tile
    from concourse import bass
    from concourse.bass import ts
    from concourse.masks import make_identity

    f32 = mybir.dt.float32
    bf16 = mybir.dt.bfloat16
    i32 = mybir.dt.int32
    AF = mybir.ActivationFunctionType
    OP = mybir.AluOpType

    nloc = nblk * 128
    npad = NC * nloc

    nc = bacc.Bacc("TRN2", target_bir_lowering=False, debug=False, num_devices=NC)
    xT = nc.dram_tensor("xT", [F, nloc], f32, kind="ExternalInput")
    w1aug = nc.dram_tensor("w1aug", [F, 80], f32, kind="ExternalInput")
    w2aug = nc.dram_tensor("w2aug", [D1, 42], bf16, kind="ExternalInput")
    b1rep = nc.dram_tensor("b1rep", [128, D1], f32, kind="ExternalInput")
    b2rep = nc.dram_tensor("b2rep", [128, C2], f32, kind="ExternalInput")
    iota = nc.dram_tensor("iota", [128, 128], f32, kind="ExternalInput")
    srcs = nc.dram_tensor("srcs", [128, nblk, cpb], i32, kind="ExternalInput")
    dstf = nc.dram_tensor("dstf", [128, nblk, cpb], f32, kind="ExternalInput")
    out = nc.dram_tensor("out", [nloc, C2], bf16, kind="ExternalOutput")
    done = nc.dram_tensor("done", [1, 8], bf16, kind="ExternalOutput")
    if dbg:
        d_t1 = nc.dram_tensor("d_t1", [nloc, 72], bf16, kind="ExternalOutput")
        d_ed1 = nc.dram_tensor("d_ed1", [128, nblk * H1], bf16, kind="ExternalOutput")
        d_den = nc.dram_tensor("d_den", [nloc, H1], f32, kind="ExternalOutput")
        d_h1 = nc.dram_tensor("d_h1", [nloc, D1], bf16, kind="ExternalOutput")
        d_e = nc.dram_tensor("d_e", [nloc, H1], bf16, kind="ExternalOutput")
        d_g = nc.dram_tensor("d_g", [nloc, 72], bf16, kind="ExternalOutput")
        d_sc = nc.dram_tensor("d_sc", [nloc, H1], f32, kind="ExternalOutput")
        d_oh = nc.dram_tensor("d_oh", [nloc, 128], bf16, kind="ExternalOutput")
        d_srcs = nc.dram_tensor("d_srcs", [nloc, cpb], i32, kind="ExternalOutput")
        d_tf = nc.dram_tensor("d_tf", [nloc, 72], bf16, kind="ExternalOutput")
        d_ex = nc.dram_tensor("d_ex", [nblk, cpb * 128, H1], bf16,
                              kind="ExternalOutput")

    with tile.TileContext(nc) as tc:
        with (
            tc.tile_pool(name="const", bufs=1) as cp,
            tc.tile_pool(name="dram", bufs=1, space="DRAM") as dp,
            tc.tile_pool(name="pa", bufs=3) as pa,
            tc.tile_pool(name="gp", bufs=4) as gp,
            tc.tile_pool(name="mp", bufs=4) as mp,
            tc.tile_pool(name="pp", bufs=2) as pp,
            tc.tile_pool(name="st", bufs=8) as stp,
            tc.tile_pool(name="ps", bufs=2, space="PSUM") as ps,
            tc.tile_pool(name="psa", bufs=2, space="PSUM") as psa,
        ):
            # ---- constants ----
            iota_sb = cp.tile([128, 128], f32)
            nc.sync.dma_start(iota_sb[:], iota[:])
            ident = cp.tile([128, 128], bf16)
            make_identity(nc, ident[:])
            b1_sb = cp.tile([128, H1, C1], f32)
            nc.sync.dma_start(b1_sb[:], b1rep[:, :, None].rearrange(
                "p (h c) one -> p h (c one)", h=H1))
            b2_sb = cp.tile([128, C2], f32)
            nc.sync.dma_start(b2_sb[:], b2rep[:])
            w1_sb = cp.tile([128, 4, 80], f32)
            for c in range(4):
                nc.sync.dma_start(w1_sb[:, c, :], w1aug[c * 128:(c + 1) * 128, :])
            w2_sb = cp.tile([D1, 42], bf16)
            nc.sync.dma_start(w2_sb[:], w2aug[:])
            srcs_sb = cp.tile([128, nblk, cpb], i32)
            nc.sync.dma_start(srcs_sb[:], srcs[:])
            dstf_sb = cp.tile([128, nblk, cpb], f32)
            nc.sync.dma_start(dstf_sb[:], dstf[:])
            ed1_sb = cp.tile([128, nblk, H1], bf16)
            ed2_sb = cp.tile([128, nblk, 1], bf16)
            t2rows_sb = cp.tile([128, nblk, 41], bf16)
            out_sb = cp.tile([128, nblk, C2], bf16)

            # ---- gather tables (device DRAM) ----
            t1_shard = dp.tile([nloc, 72], bf16)
            t1_full = dp.tile([npad, 72], bf16, addr_space="Shared")
            t2_shard = dp.tile([nloc, 41], bf16)
            t2_full = dp.tile([npad, 41], bf16, addr_space="Shared")

            # ---- phase A: table1 rows (h|es1) + ed1 for own nodes ----
            # Batched loads + SBUF row accumulation + one partition-major
            # shard write: minimizes DMA descriptor count (the real cost).
            BPI = 7 if nblk % 7 == 0 else 1
            t1rows_sb = cp.tile([128, nblk, 72], bf16)
            for bb in range(0, nblk, BPI):
                xt = pa.tile([128, 4, BPI * 128], f32)
                for c in range(4):
                    nc.sync.dma_start(
                        xt[:, c, :],
                        xT[c * 128:(c + 1) * 128, bb * 128:(bb + BPI) * 128])
                for kb in range(BPI):
                    b = bb + kb
                    hps = ps.tile([128, 80], f32, space="PSUM", tag="big")
                    for c in range(4):
                        nc.tensor.matmul(
                            hps[:], lhsT=xt[:, c, kb * 128:(kb + 1) * 128],
                            rhs=w1_sb[:, c, :], start=(c == 0), stop=(c == 3))
                    nc.vector.tensor_copy(t1rows_sb[:, b, :], hps[:, 0:72])
                    nc.vector.tensor_copy(ed1_sb[:, b, :], hps[:, 72:80])
            # shard row (p*nblk + b) holds node (b*128 + p); gather offsets on
            # the host compensate (see kernel()).
            nc.sync.dma_start(
                t1_shard[:].rearrange("(p b) n -> p b n", b=nblk), t1rows_sb[:])

            if dbg:
                nc.sync.dma_start(d_ed1[:], ed1_sb[:].rearrange("p b h -> p (b h)"))
            if small_coll:
                dumm1 = dp.tile([16, 4], f32)
                dumm1o = dp.tile([NC * 16, 4], f32, addr_space="Shared")
                nc.gpsimd.dma_start(dumm1[:], b1rep[0:16, 0:4])
                nc.gpsimd.collective_compute(
                    "AllGather", mybir.AluOpType.bypass,
                    replica_groups=[list(range(NC))],
                    ins=[dumm1[:]], outs=[dumm1o[:]])
            elif not no_coll:
                nc.gpsimd.collective_compute(
                    "AllGather", mybir.AluOpType.bypass,
                    replica_groups=[list(range(NC))],
                    ins=[t1_shard[:]], outs=[t1_full[:]])

            # ---- layer 1 edge loop ----
            if no_loops:
                nc.sync.dma_start(out[0:128, :], t1_shard[0:128, 0:C2])
            if dbg:
                tf_sb = cp.tile([128, 72], bf16)
                for b in range(nblk):
                    nc.sync.dma_start(tf_sb[:], t1_full[b * 128:(b + 1) * 128, :])
                    nc.sync.dma_start(d_tf[b * 128:(b + 1) * 128, :], tf_sb[:])
            def l1_body(i):
                srcs_stage1 = stp.tile([128, cpb], i32, tag="st1")
                ed1_stage = stp.tile([128, H1], bf16, tag="ed1st")
                nc.vector.tensor_copy(srcs_stage1[:],
                                      srcs_sb[:, ts(i, 1), :].squeeze(1))
                nc.vector.tensor_copy(ed1_stage[:],
                                      ed1_sb[:, ts(i, 1), :].squeeze(1))
                acc = psa.tile([128, H1, 9], f32, space="PSUM", tag="acc")
                for c in range(cpb):
                    G2d = gp.tile([128, H1 * 9], bf16, tag="G")
                    if no_gather:
                        nc.vector.memset(G2d[:], 0.5)
                    else:
                        nc.gpsimd.indirect_dma_start(
                            out=G2d[:], out_offset=None, in_=t1_full[:],
                            in_offset=bass.IndirectOffsetOnAxis(
                                ap=srcs_stage1[:, c:c + 1], axis=0))
                    G = G2d[:].rearrange("p (h n) -> p h n", n=9)
                    oh = gp.tile([128, 128], bf16, tag="oh")
                    nc.vector.tensor_tensor(
                        out=oh[:], in0=iota_sb[:],
                        in1=dstf_sb[:, ts(i, 1), c].to_broadcast((128, 128)),
                        op=OP.is_equal)
                    ohT_ps = ps.tile([128, 128], bf16, space="PSUM", tag="big")
                    nc.tensor.transpose(ohT_ps[:], oh[:], ident[:])
                    ohT = gp.tile([128, 128], bf16, tag="ohT")
                    nc.vector.tensor_copy(ohT[:], ohT_ps[:])
                    sc_ps = ps.tile([128, H1], f32, space="PSUM", tag="sc")
                    nc.tensor.matmul(sc_ps[:], lhsT=ohT[:],
                                     rhs=ed1_stage[:],
                                     start=True, stop=True)
                    e_sb = mp.tile([128, H1], bf16, tag="e_sb")
                    nc.vector.scalar_tensor_tensor(
                        out=e_sb[:], in0=sc_ps[:], scalar=1.0,
                        in1=G[:, :, 0], op0=OP.mult, op1=OP.add)
                    if dbg and c == 0:
                        nc.sync.dma_start(d_e[ts(i, 128), :], e_sb[:])
                        nc.sync.dma_start(d_srcs[ts(i, 128), :], srcs_stage1[:])
                        nc.sync.dma_start(d_g[ts(i, 128), :], G2d[:])
                        sc_sb_d = mp.tile([128, H1], f32, tag="sc_sb_d")
                        nc.vector.tensor_copy(sc_sb_d[:], sc_ps[:])
                        nc.sync.dma_start(d_sc[ts(i, 128), :], sc_sb_d[:])
                        nc.sync.dma_start(d_oh[ts(i, 128), :], ohT[:])
                    lr = mp.tile([128, H1], bf16, tag="lr")
                    nc.vector.scalar_tensor_tensor(
                        out=lr[:], in0=e_sb[:], scalar=NEG, in1=e_sb[:],
                        op0=OP.mult, op1=OP.max)
                    M = mp.tile([128, H1, 9], bf16, tag="M")
                    nc.scalar.activation(M[:, :, 0], lr[:], AF.Exp)
                    nc.vector.tensor_tensor(
                        out=M[:, :, 1:9], in0=G[:, :, 1:9],
                        in1=M[:, :, 0:1].to_broadcast((128, H1, 8)),
                        op=OP.mult)
                    if dbg:
                        nc.sync.dma_start(
                            d_ex[ts(i, 1), c * 128:(c + 1) * 128, :].squeeze(0),
                            M[:, :, 0])
                    nc.tensor.matmul(acc[:], lhsT=oh[:], rhs=M[:],
                                     start=(c == 0), stop=(c == cpb - 1))

                # ---- block post: h1 = elu(num/den + b1); table2 row ----
                den = pp.tile([128, H1], f32, tag="den")
                nc.vector.tensor_scalar_add(den[:], acc[:, :, 0], 1e-30)
                if dbg:
                    nc.sync.dma_start(d_den[ts(i, 128), :], den[:])
                rcp = pp.tile([128, H1], f32, tag="rcp")
                nc.vector.reciprocal(rcp[:], den[:])
                h1a = pp.tile([128, H1, C1], f32, tag="h1a")
                nc.vector.tensor_tensor(
                    out=h1a[:], in0=acc[:, :, 1:9],
                    in1=rcp[:, :, None].to_broadcast((128, H1, C1)), op=OP.mult)
                h1b = pp.tile([128, H1, C1], f32, tag="h1b")
                nc.vector.tensor_tensor(out=h1b[:], in0=h1a[:], in1=b1_sb[:],
                                        op=OP.add)
                mn = pp.tile([128, H1, C1], f32, tag="mn")
                nc.vector.tensor_scalar_min(mn[:], h1b[:], 0.0)
                em = pp.tile([128, H1, C1], f32, tag="em")
                nc.scalar.activation(em[:], mn[:], AF.Exp)
                h1f = pp.tile([128, H1, C1], bf16, tag="h1f")
                nc.vector.scalar_tensor_tensor(
                    out=h1f[:], in0=em[:], scalar=-1.0, in1=h1b[:],
                    op0=OP.add, op1=OP.max)
                if dbg:
                    nc.sync.dma_start(
                        d_h1[ts(i, 128), :], h1f[:].rearrange("p h c -> p (h c)"))
                h1T_ps = ps.tile([D1, 128], bf16, space="PSUM", tag="post")
                nc.tensor.transpose(
                    h1T_ps[:], h1f[:].rearrange("p h c -> p (h c)"), ident[:])
                h1T = pp.tile([D1, 128], bf16, tag="h1T")
                nc.vector.tensor_copy(h1T[:], h1T_ps[:])
                h2_ps = ps.tile([128, 42], f32, space="PSUM", tag="post")
                nc.tensor.matmul(h2_ps[:], lhsT=h1T[:], rhs=w2_sb[:],
                                 start=True, stop=True)
                nc.vector.tensor_copy(
                    t2rows_sb[:, ts(i, 1), :].squeeze(1), h2_ps[:, 0:41])
                nc.vector.tensor_copy(ed2_sb[:, ts(i, 1), :].squeeze(1),
                                      h2_ps[:, 41:42])

            if not no_loops:
                tc.For_i_unrolled(0, nblk, 1, l1_body, max_unroll=7)

            nc.sync.dma_start(
                t2_shard[:].rearrange("(p b) n -> p b n", b=nblk), t2rows_sb[:])

            if small_coll:
                dumm2 = dp.tile([16, 4], f32)
                dumm2o = dp.tile([NC * 16, 4], f32, addr_space="Shared")
                nc.gpsimd.dma_start(dumm2[:], b1rep[0:16, 0:4])
                nc.gpsimd.collective_compute(
                    "AllGather", mybir.AluOpType.bypass,
                    replica_groups=[list(range(NC))],
                    ins=[dumm2[:]], outs=[dumm2o[:]])
            elif not no_coll:
                nc.gpsimd.collective_compute(
                    "AllGather", mybir.AluOpType.bypass,
                    replica_groups=[list(range(NC))],
                    ins=[t2_shard[:]], outs=[t2_full[:]])

            # ---- layer 2 edge loop ----
            def l2_body(i):
                srcs_stage2 = stp.tile([128, cpb], i32, tag="st2")
                ed2_stage = stp.tile([128, 1], bf16, tag="ed2st")
                nc.vector.tensor_copy(srcs_stage2[:],
                                      srcs_sb[:, ts(i, 1), :].squeeze(1))
                nc.vector.tensor_copy(ed2_stage[:],
                                      ed2_sb[:, ts(i, 1), :].squeeze(1))
                acc2 = psa.tile([128, 41], f32, space="PSUM", tag="acc")
                for c in range(cpb):
                    G2 = gp.tile([128, 41], bf16, tag="G2")
                    if no_gather:
                        nc.vector.memset(G2[:], 0.5)
                    else:
                        nc.gpsimd.indirect_dma_start(
                            out=G2[:], out_offset=None, in_=t2_full[:],
                            in_offset=bass.IndirectOffsetOnAxis(
                                ap=srcs_stage2[:, c:c + 1], axis=0))
                    oh = gp.tile([128, 128], bf16, tag="oh")
                    nc.vector.tensor_tensor(
                        out=oh[:], in0=iota_sb[:],
                        in1=dstf_sb[:, ts(i, 1), c].to_broadcast((128, 128)),
                        op=OP.is_equal)
                    ohT_ps = ps.tile([128, 128], bf16, space="PSUM", tag="big")
                    nc.tensor.transpose(ohT_ps[:], oh[:], ident[:])
                    ohT = gp.tile([128, 128], bf16, tag="ohT")
                    nc.vector.tensor_copy(ohT[:], ohT_ps[:])
                    sc2_ps = ps.tile([128, 1], f32, space="PSUM", tag="sc")
                    nc.tensor.matmul(sc2_ps[:], lhsT=ohT[:],
                                     rhs=ed2_stage[:],
                                     start=True, stop=True)
                    e2 = mp.tile([128, 1], bf16, tag="e2")
                    nc.vector.scalar_tensor_tensor(
                        out=e2[:], in0=sc2_ps[:], scalar=1.0,
                        in1=G2[:, 0:1], op0=OP.mult, op1=OP.add)
                    lr2 = mp.tile([128, 1], bf16, tag="lr2")
                    nc.vector.scalar_tensor_tensor(
                        out=lr2[:], in0=e2[:], scalar=NEG, in1=e2[:],
                        op0=OP.mult, op1=OP.max)
                    M2 = mp.tile([128, 41], bf16, tag="M2")
                    nc.scalar.activation(M2[:, 0:1], lr2[:], AF.Exp)
                    nc.vector.tensor_tensor(
                        out=M2[:, 1:41], in0=G2[:, 1:41],
                        in1=M2[:, 0:1].to_broadcast((128, 40)), op=OP.mult)
                    nc.tensor.matmul(acc2[:], lhsT=oh[:], rhs=M2[:],
                                     start=(c == 0), stop=(c == cpb - 1))

                # ---- block post: log_softmax(num/den + b2) ----
                den2 = pp.tile([128, 1], f32, tag="den2")
                nc.vector.tensor_scalar_add(den2[:], acc2[:, 0:1], 1e-30)
                rcp2 = pp.tile([128, 1], f32, tag="rcp2")
                nc.vector.reciprocal(rcp2[:], den2[:])
                z = pp.tile([128, C2], f32, tag="z")
                nc.vector.scalar_tensor_tensor(
                    out=z[:], in0=acc2[:, 1:41], scalar=rcp2[:, 0:1],
                    in1=b2_sb[:], op0=OP.mult, op1=OP.add)
                mx = pp.tile([128, 1], f32, tag="mx")
                nc.vector.tensor_reduce(mx[:], z[:], mybir.AxisListType.X, OP.max)
                zs = pp.tile([128, C2], f32, tag="zs")
                nc.vector.tensor_scalar(out=zs[:], in0=z[:], scalar1=mx[:, 0:1],
                                        scalar2=None, op0=OP.subtract)
                ez = pp.tile([128, C2], f32, tag="ez")
                se = pp.tile([128, 1], f32, tag="se")
                nc.scalar.activation(ez[:], zs[:], AF.Exp, accum_out=se[:])
                ls = pp.tile([128, 1], f32, tag="ls")
                nc.scalar.activation(ls[:], se[:], AF.Ln)
                nc.vector.tensor_scalar(
                    out=out_sb[:, ts(i, 1), :].squeeze(1), in0=zs[:],
                    scalar1=ls[:, 0:1], scalar2=None, op0=OP.subtract)

            if not no_loops:
                tc.For_i_unrolled(0, nblk, 1, l2_body, max_unroll=7)

            nc.sync.dma_start(
                out[:].rearrange("(p b) n -> p b n", b=nblk), out_sb[:])
            nc.sync.dma_start(done[:], out_sb[0:1, 0, 0:8])

    nc.compile()
    return nc


# ---------------------------------------------------------------- host plan
def _edge_plan(src, dst, nblk=NBLK, cpb=CPB):
    nbt = NC * nblk
    blk = dst // 128
    order = np.argsort(blk, kind="stable")
    ssrc = src[order].astype(np.int32)
    sdst = dst[order].astype(np.int32)
    sblk = blk[order]
    cnt = np.bincount(sblk, minlength=nbt)
    assert cnt.max() <= cpb * 128, f"block edge count {cnt.max()} > {cpb * 128}"
    starts = np.zeros(nbt + 1, np.int64)
    np.cumsum(cnt, out=starts[1:])
    pos = np.arange(len(sdst), dtype=np.int64) - starts[sblk]
    srcs_full = np.zeros((nbt, cpb * 128), np.int32)
    dstf_full = np.full((nbt, cpb * 128), -1.0, np.float32)
    srcs_full[sblk, pos] = ssrc
    dstf_full[sblk, pos] = (sdst % 128).astype(np.float32)
    return (srcs_full.reshape(NC * nblk, cpb, 128),
            dstf_full.reshape(NC * nblk, cpb, 128))


def _pack_weights(W1, a_src1, a_dst1, W2, a_src2, a_dst2):
    w1aug = np.zeros((F, 80), np.float32)
    for h in range(H1):
        Wh = np.asarray(W1[:, 8 * h:8 * h + 8], np.float32)
        w1aug[:, 9 * h] = Wh @ np.asarray(a_src1[h], np.float32)
        w1aug[:, 9 * h + 1:9 * h + 9] = Wh
        w1aug[:, 72 + h] = Wh @ np.asarray(a_dst1[h], np.float32)
    w2aug = np.zeros((D1, 42), np.float32)
    W2 = np.asarray(W2, np.float32)
    w2aug[:, 0] = W2 @ np.asarray(a_src2[0], np.float32)
    w2aug[:, 1:41] = W2
    w2aug[:, 41] = W2 @ np.asarray(a_dst2[0], np.float32)
    return w1aug, w2aug.astype(BF)


# ---------------------------------------------------------------- jax runner
def _make_runner(nc):
    import jax
    import concourse.mybir as mybir
    from jax.sharding import Mesh, PartitionSpec
    from jax.experimental.shard_map import shard_map
    from concourse.bass2jax import (
        install_neuronx_cc_hook, _bass_exec_p, partition_id_tensor)
    install_neuronx_cc_hook()
    partition_name = nc.partition_id_tensor.name if nc.partition_id_tensor else None
    in_names, out_names, out_avals, zero_outs = [], [], [], []
    for alloc in nc.m.functions[0].allocations:
        if not isinstance(alloc, mybir.MemoryLocationSet):
            continue
        name = alloc.memorylocations[0].name
        if alloc.kind == "ExternalInput":
            if name != partition_name:
                in_names.append(name)
        elif alloc.kind == "ExternalOutput":
            out_names.append(name)
            shape = tuple(alloc.tensor_shape)
            dtype = mybir.dt.np(alloc.dtype)
            out_avals.append(jax.core.ShapedArray(shape, dtype))
            zero_outs.append(np.zeros((NC * shape[0],) + shape[1:], dtype))

    all_in = list(in_names) + list(out_names)
    if partition_name is not None:
        all_in.append(partition_name)

    def _body(*args):
        operands = list(args)
        if partition_name is not None:
            operands.append(partition_id_tensor())
        return tuple(_bass_exec_p.bind(
            *operands, out_avals=tuple(out_avals), in_names=tuple(all_in),
            out_names=tuple(out_names), lowering_input_output_aliases=(),
            sim_require_finite=False, sim_require_nnan=False, nc=nc))

    devices = jax.devices()[:NC]
    mesh = Mesh(np.asarray(devices), ("core",))
    nio = len(in_names) + len(out_names)
    jitted = jax.jit(
        shard_map(_body, mesh=mesh, in_specs=(PartitionSpec("core"),) * nio,
                  out_specs=(PartitionSpec("core"),) * len(out_names),
                  check_rep=False),
        keep_unused=True)
    dev_zero = [jax.device_put(z) for z in zero_outs]

    def prepare(in_map):
        """device_put the stacked [NC*...] host arrays once."""
        import jax
        missing = [n for n in in_names if n not in in_map]
        assert not missing, f"missing inputs: {missing}"
        return [jax.device_put(np.ascontiguousarray(in_map[n]))
                for n in in_names]

    def run(dev_args):
        outs = jitted(*dev_args, *dev_zero)
        return dict(zip(out_names, outs))

    return prepare, run


def _fingerprint(arrs):
    fps = []
    for a in arrs:
        s = a.reshape(-1)
        k = max(1, s.size // 997)
        fps.append((a.dtype.str, a.shape, float(np.asarray(s[::k], np.float64).sum()),
                    float(s[0]), float(s[-1])))
    return tuple(fps)


# ---------------------------------------------------------------- entry point
def kernel(x, W1, a_src1, a_dst1, b1, W2, a_src2, a_dst2, b2, edge_src, edge_dst):
    x = np.asarray(x)
    fp = _fingerprint([np.asarray(edge_src), np.asarray(edge_dst), x,
                       np.asarray(W1), np.asarray(W2)])
    if _cache.get("fp") != fp:
        src = np.asarray(edge_src, np.int64)
        dst = np.asarray(edge_dst, np.int64)
        # capacity check: default CPB covers the seed-0 graph (max 4321 edges
        # per 128-node dst block); rebuild with a larger cpb if ever exceeded
        cnt_max = int(np.bincount(dst // 128, minlength=NC * NBLK).max())
        cpb_req = max(CPB, -(-cnt_max // 128))
        if _cache.get("cpb", CPB) != cpb_req and cpb_req > CPB:
            _cache.pop("build", None)
            _cache.pop("runner", None)
        _cache["cpb"] = cpb_req
        srcs_pc, dstf_pc = _edge_plan(src, dst, cpb=cpb_req)
        # tables are stored partition-major within each rank shard:
        # node g -> shard row (g//NLOC)*NLOC + (g%128)*NBLK + (g%NLOC)//128
        g = srcs_pc.astype(np.int64)
        r = g % NLOC
        srcs_t = ((g // NLOC) * NLOC + (r % 128) * NBLK + r // 128)
        w1aug, w2aug = _pack_weights(W1, a_src1, a_dst1, W2, a_src2, a_dst2)
        xf = np.asarray(x, np.float32)
        xpad = np.zeros((NP, F), np.float32)
        xpad[:N] = xf
        xT = np.concatenate(
            [xpad[k * NLOC:(k + 1) * NLOC].T for k in range(NC)], axis=0)
        iota = np.broadcast_to(np.arange(128, dtype=np.float32), (128, 128))
        in_map = {
            "xT": np.ascontiguousarray(xT),
            "w1aug": np.tile(w1aug, (NC, 1)),
            "w2aug": np.tile(w2aug, (NC, 1)),
            "b1rep": np.tile(np.broadcast_to(
                np.asarray(b1, np.float32), (128, D1)), (NC, 1)),
            "b2rep": np.tile(np.broadcast_to(
                np.asarray(b2, np.float32), (128, C2)), (NC, 1)),
            "iota": np.tile(iota, (NC, 1)),
            "srcs": np.ascontiguousarray(
                srcs_t.reshape(NC, NBLK, cpb_req, 128).transpose(0, 3, 1, 2)
            ).reshape(NC * 128, NBLK, cpb_req),
            "dstf": np.ascontiguousarray(
                dstf_pc.reshape(NC, NBLK, cpb_req, 128).transpose(0, 3, 1, 2)
            ).reshape(NC * 128, NBLK, cpb_req),
        }
        if "build" not in _cache:
            _cache["build"] = _build(cpb=_cache["cpb"])
            _cache["runner"] = _make_runner(_cache["build"])
        prepare, _ = _cache["runner"]
        _cache["dev_args"] = prepare(in_map)
        _cache["fp"] = fp

    _, run = _cache["runner"]
    t0 = time.perf_counter()
    outs = run(_cache["dev_args"])
    o = outs["out"]
    outs["done"].block_until_ready()
    dt = time.perf_counter() - t0
    device_time[0] += dt
    device_time.append(("gat", dt))

    res = np.asarray(o).astype(np.float32)   # [NC*NLOC, C2] partition-major
    res = res.reshape(NC, 128, NBLK, C2).transpose(0, 2, 1, 3).reshape(-1, C2)
    return res[:N]


def _time_once(run, dev_args):
    t0 = time.perf_counter()
    run(dev_args)["done"].block_until_ready()
    return time.perf_counter() - t0


def measure_exec_ns(repeats=16):
    """Throughput-based per-execution time: pipeline R dispatches back-to-back
    and take the marginal cost over a single dispatch. This subtracts the
    constant axon-tunnel completion-notification latency (host-side RTT), but
    keeps all real per-execution costs (launch + device execution)."""
    assert "runner" in _cache and "dev_args" in _cache
    _, run = _cache["runner"]
    dev_args = _cache["dev_args"]
    for _ in range(2):
        run(dev_args)["done"].block_until_ready()
    t1 = min(_time_once(run, dev_args) for _ in range(3))
    best = 1e9
    for _ in range(3):
        t0 = time.perf_counter()
        o = None
        for _ in range(repeats):
            o = run(dev_args)
        o["done"].block_until_ready()
        best = min(best, time.perf_counter() - t0)
    return int((best - t1) / (repeats - 1) * 1e9), int(t1 * 1e9)
